# revision 15
# baseline (speedup 1.0000x reference)
"""Bahdanau attention on TRN2 — data-parallel over batch across 8 NeuronCores.

Math per batch row n (shapes: T=2048 encoder steps, E=U=1024):
    K_projT[u, t] = sum_e Wk[u, e] * X[n, t, e]          (big matmul, [U, T] layout)
    th[u, t]      = tanh(K_projT[u, t] + q_proj[n, u])   (ACT, per-partition bias)
    scores[t]     = sum_u v[u] * th[u, t]                (PE, v as 1-col stationary)
    a[t]          = softmax(scores + mask[n])            (mask additive -1e30)
    ctx[e]        = sum_t a[t] * X[n, t, e]              (PE, aT cols as stationary)

Host precomputes q_proj = queries @ Wq.T (tiny), the additive mask from
`lengths`, X transposed per row ([E, T]) so the contraction dim lands on
SBUF partitions, plus small layout shuffles of Wk / v / q_proj.
"""

import numpy as np

import concourse.bass as bass
import concourse.mybir as mybir
import concourse.tile as tile
from concourse.bass_utils import run_bass_kernel_spmd

# Problem shape (hardcoded per contract; kernel.py must be self-contained).
N, T, D_ENC, D_DEC, U = 32, 2048, 1024, 1024, 1024
N_CORES = 8
R = N // N_CORES            # batch rows per core
P = 128                     # SBUF partitions
TC = 512                    # t-chunk = matmul moving free dim (fp32 max)
NTC = T // TC
ET = D_ENC // P             # e-tiles (contraction of the big matmul)
UT = U // P                 # u-tiles
TT = T // P                 # t-tiles (contraction of the context matmul)
EC = 512
NEC = D_ENC // EC

F32 = mybir.dt.float32
# PE matmul dtype. float32r = single-pass fp32 matmul (full rate at free
# dim >= 256); plain float32 = 2 half-speed passes (4x slower).
MM_DT = mybir.dt.float32r

AF = mybir.ActivationFunctionType
AX = mybir.AxisListType

MASK_NEG = np.float32(-1.0e30)

LAST_RESULTS = None         # BassKernelResults of the most recent run
_PROGRAM = None


def _mm(ap):
    return ap if ap.dtype == MM_DT else ap.bitcast(MM_DT)


def _legalize_waits(nc):
    """Several walrus instruction encodings (the self-loading fp32r matmul's
    S3_LW, Activation's S3D3_AC, ...) have a single sync-wait slot, but Tile
    sometimes emits 2+ waits on one instruction. Hoist the extra waits onto
    engine NoOps inserted just before the instruction — the engine's NX
    evaluates waits in program order, so gating is preserved. This covers
    HWDGE DMAs too: the issuing engine's sequencer writes the descriptor
    in program order, so a same-engine NoOp gates the transfer."""
    for f in nc.m.functions:
        for blk in f.blocks:
            insts = blk.instructions
            idx = 0
            while idx < len(insts):
                ins = insts[idx]
                if (
                    not isinstance(ins, mybir.InstCollectiveCompute)
                    and ins.engine is not None
                    and ins.sync_info is not None
                    and len(ins.sync_info.on_wait) > 1
                ):
                    waits = list(ins.sync_info.on_wait)
                    # one wait per NoOp — every ISA ctrl struct fits that
                    for w in waits[1:]:
                        nop = mybir.InstNoOp(
                            name=nc.get_next_instruction_name(), ins=[], outs=[]
                        )
                        nop.engine = ins.engine
                        nop.sync_info = mybir.SyncInfo(on_wait=[w], on_update=[])
                        insts.insert(idx, nop)
                        idx += 1
                    ins.sync_info = mybir.SyncInfo(
                        on_wait=[waits[0]], on_update=list(ins.sync_info.on_update)
                    )
                idx += 1


def build_program(legalize: bool = True) -> bass.Bass:
    nc = bass.Bass("TRN2")

    xt = nc.dram_tensor("xt", [R, D_ENC, T], F32, kind="ExternalInput").ap()
    xn = nc.dram_tensor("xn", [R, T, D_ENC], F32, kind="ExternalInput").ap()
    wkt = nc.dram_tensor("wkt", [P, UT, ET, P], F32, kind="ExternalInput").ap()
    vt = nc.dram_tensor("vt", [P, UT], F32, kind="ExternalInput").ap()
    qpt = nc.dram_tensor("qpt", [P, R * UT], F32, kind="ExternalInput").ap()
    mask = nc.dram_tensor("mask", [R, T], F32, kind="ExternalInput").ap()
    ctx_out = nc.dram_tensor("contexts", [R, D_ENC], F32, kind="ExternalOutput").ap()
    align_out = nc.dram_tensor("alignments", [R, T], F32, kind="ExternalOutput").ap()

    with tile.TileContext(nc) as tc:
        with (
            tc.tile_pool(name="const", bufs=1) as const_pool,
            tc.tile_pool(name="xtp", bufs=4) as xt_pool,
            tc.tile_pool(name="thp", bufs=4) as th_pool,
            tc.tile_pool(name="xnp", bufs=6) as xn_pool,
            tc.tile_pool(name="rowp", bufs=2) as row_pool,
            tc.tile_pool(name="psm", bufs=3, space="PSUM") as psum_m,
            tc.tile_pool(name="pss", bufs=2, space="PSUM") as psum_s,
            tc.tile_pool(name="psc", bufs=2, space="PSUM") as psum_c,
            tc.tile_pool(name="dramp", bufs=2, space="DRAM") as dram_pool,
        ):
            # Replicated constants. wk is loaded per-ut-slice (512 KB each) so
            # the first matmul group only waits for 1/8 of Wk; the first xt
            # chunk's DMAs are emitted right after wk[ut=0] so compute starts
            # ~7 us in instead of waiting for all 6 MB of startup traffic.
            wk_sb = const_pool.tile([P, UT, ET, P], MM_DT, tag="wk")
            nc.sync.dma_start(wk_sb[:, 0], _mm(wkt[:, 0]))

            def load_xt_chunk(r, c):
                xt_sb = xt_pool.tile([P, ET, TC], MM_DT, tag="xt")
                for et in range(ET):
                    nc.sync.dma_start(
                        xt_sb[:, et],
                        _mm(xt[r, et * P:(et + 1) * P, c * TC:(c + 1) * TC]),
                    )
                return xt_sb

            xt_first = load_xt_chunk(0, 0)
            for ut in range(1, UT):
                nc.sync.dma_start(wk_sb[:, ut], _mm(wkt[:, ut]))
            vt_sb = const_pool.tile([P, UT], MM_DT, tag="vt")
            nc.sync.dma_start(vt_sb[:], _mm(vt[:, :]))
            qpt_sb = const_pool.tile([P, R * UT], F32, tag="qpt")
            nc.sync.dma_start(qpt_sb[:], qpt[:, :])

            row_state = {}

            def phase_a_chunk(r, c, xt_sb, row):
                sc_ps = psum_s.tile([1, TC], F32, tag="sc")
                ths = []
                for ut in range(UT):
                    ps = psum_m.tile([P, TC], F32, tag="kproj")
                    for et in range(ET):
                        nc.tensor.matmul(
                            ps[:],
                            wk_sb[:, ut, et],
                            xt_sb[:, et],
                            start=(et == 0),
                            stop=(et == ET - 1),
                        )
                    th = th_pool.tile([P, TC], MM_DT, tag="th")
                    nc.scalar.activation(
                        th[:], ps[:], AF.Tanh,
                        bias=qpt_sb[:, r * UT + ut:r * UT + ut + 1],
                    )
                    ths.append(th)
                    # score matmul for ut-1: one main group behind, so the
                    # tanh it waits on is already finished (no PE stall).
                    if ut > 0:
                        nc.tensor.matmul(
                            sc_ps[:], vt_sb[:, ut - 1:ut], _mm(ths[ut - 1][:]),
                            start=(ut == 1), stop=False,
                        )
                nc.tensor.matmul(
                    sc_ps[:], vt_sb[:, UT - 1:UT], _mm(ths[UT - 1][:]),
                    start=False, stop=True,
                )
                nc.vector.tensor_add(
                    row["scores"][:, c * TC:(c + 1) * TC], sc_ps[:],
                    row["mask"][:, c * TC:(c + 1) * TC],
                )
                # per-chunk max, so the end-of-row reduction is tiny and the
                # PE gap before the context matmuls stays under the HAM
                # re-throttle window.
                nc.vector.reduce_max(
                    row["mx4"][:, c:c + 1],
                    row["scores"][:, c * TC:(c + 1) * TC], axis=AX.X,
                )

            def softmax_row(r, row):
                mxn = row_pool.tile([1, 1], F32, tag="mxn")
                nc.vector.reduce_max(mxn[:], row["mx4"][:], axis=AX.X, negate=True)
                exp_sb = row_pool.tile([1, T], F32, tag="exp")
                zsum = row_pool.tile([1, 1], F32, tag="z")
                nc.scalar.activation(
                    exp_sb[:], row["scores"][:], AF.Exp, bias=mxn[:],
                    accum_out=zsum[:],
                )
                rz = row_pool.tile([1, 1], F32, tag="rz")
                nc.vector.reciprocal(rz[:], zsum[:])
                # aT via DRAM bounce (unnormalized; 1/Z folded into outputs)
                bounce = dram_pool.tile([1, T], F32, tag="bounce")
                nc.sync.dma_start(bounce[:], exp_sb[:])
                at_sb = row_pool.tile([P, TT], MM_DT, tag="at")
                nc.sync.dma_start(
                    at_sb[:], _mm(bounce[0].rearrange("(j p) -> p j", p=P))
                )
                align_sb = row_pool.tile([1, T], F32, tag="align")
                nc.scalar.activation(align_sb[:], exp_sb[:], AF.Copy, scale=rz[:])
                nc.sync.dma_start(align_out[r:r + 1, :], align_sb[:])
                row["at"], row["rz"] = at_sb, rz

            def phase_b_row(r, row):
                # ctx[e] = (1/Z) * sum_t exp[t] * X[t, e]
                ct_ps0 = psum_c.tile([1, EC], F32, tag="ctx")
                ct_ps1 = psum_c.tile([1, EC], F32, tag="ctx")
                at_sb, rz = row["at"], row["rz"]
                for tt in range(TT):
                    xn_sb = xn_pool.tile([P, D_ENC], MM_DT, tag="xn")
                    nc.sync.dma_start(xn_sb[:], _mm(xn[r, tt * P:(tt + 1) * P, :]))
                    nc.tensor.matmul(
                        ct_ps0[:], at_sb[:, tt:tt + 1], xn_sb[:, 0:EC],
                        start=(tt == 0), stop=(tt == TT - 1),
                    )
                    nc.tensor.matmul(
                        ct_ps1[:], at_sb[:, tt:tt + 1], xn_sb[:, EC:2 * EC],
                        start=(tt == 0), stop=(tt == TT - 1),
                    )
                ctx_sb = row_pool.tile([1, D_ENC], F32, tag="ctx_sb")
                nc.scalar.activation(ctx_sb[:, 0:EC], ct_ps0[:], AF.Copy, scale=rz[:])
                nc.scalar.activation(ctx_sb[:, EC:], ct_ps1[:], AF.Copy, scale=rz[:])
                nc.sync.dma_start(ctx_out[r:r + 1, :], ctx_sb[:])

            for r in range(R):
                mask_sb = row_pool.tile([1, T], F32, tag="mask")
                nc.sync.dma_start(mask_sb[:], mask[r:r + 1, :])
                row = {
                    "mask": mask_sb,
                    "scores": row_pool.tile([1, T], F32, tag="scores", name="scores_sb"),
                    "mx4": row_pool.tile([1, NTC], F32, tag="mx4", name="mx4_sb"),
                }
                row_state[r] = row
                for c in range(NTC):
                    xt_sb = xt_first if (r == 0 and c == 0) else load_xt_chunk(r, c)
                    phase_a_chunk(r, c, xt_sb, row)
                    # previous row's context matmuls slot in here, one chunk
                    # deep into this row, so PE never stalls on its softmax.
                    if r > 0 and c == 1:
                        phase_b_row(r - 1, row_state[r - 1])
                softmax_row(r, row)
            phase_b_row(R - 1, row_state[R - 1])

    if legalize:
        _legalize_waits(nc)
    return nc


def _get_program() -> bass.Bass:
    global _PROGRAM
    if _PROGRAM is None:
        _PROGRAM = build_program()
    return _PROGRAM


def make_in_maps(queries, encoder_output, lengths, v, Wq, Wk):
    """Host-side marshalling: shard batch across cores + layout shuffles."""
    queries = np.ascontiguousarray(np.asarray(queries), dtype=np.float32)
    encoder_output = np.ascontiguousarray(np.asarray(encoder_output), dtype=np.float32)
    lengths = np.asarray(lengths).astype(np.int64)
    v = np.asarray(v, dtype=np.float32)
    Wq = np.asarray(Wq, dtype=np.float32)
    Wk = np.asarray(Wk, dtype=np.float32)

    qp = queries[:, 0, :] @ Wq.T                                   # [N, U]
    xt_full = np.ascontiguousarray(encoder_output.transpose(0, 2, 1))  # [N, E, T]
    # wkt[p, ut, et, j] = Wk[ut*128+j, et*128+p]
    wkt = np.ascontiguousarray(Wk.reshape(UT, P, ET, P).transpose(3, 0, 2, 1))
    vt = np.ascontiguousarray(v.reshape(UT, P).T)                  # [P, UT]
    mask = np.where(
        np.arange(T)[None, :] >= lengths[:, None], MASK_NEG, np.float32(0.0)
    ).astype(np.float32)                                           # [N, T]

    in_maps = []
    for i in range(N_CORES):
        sl = slice(i * R, (i + 1) * R)
        qpt = np.ascontiguousarray(
            qp[sl].reshape(R, UT, P).transpose(2, 0, 1).reshape(P, R * UT)
        )
        in_maps.append({
            "xt": xt_full[sl],
            "xn": encoder_output[sl],
            "wkt": wkt,
            "vt": vt,
            "qpt": qpt,
            "mask": np.ascontiguousarray(mask[sl]),
        })
    return in_maps


def kernel(queries, encoder_output, lengths, v, Wq, Wk, _trace=False):
    global LAST_RESULTS
    in_maps = make_in_maps(queries, encoder_output, lengths, v, Wq, Wk)
    nc = _get_program()
    res = run_bass_kernel_spmd(
        nc, in_maps, core_ids=list(range(N_CORES)), trace=_trace
    )
    LAST_RESULTS = res
    contexts = np.concatenate(
        [res.results[i]["contexts"] for i in range(N_CORES)], axis=0
    )
    alignments = np.concatenate(
        [res.results[i]["alignments"] for i in range(N_CORES)], axis=0
    )
    return contexts, alignments


# revision 17
# speedup vs baseline: 1.1557x; 1.1557x over previous
"""Bahdanau attention on TRN2 — data-parallel over batch across 8 NeuronCores.

Math per batch row n (shapes: T=2048 encoder steps, E=U=1024):
    K_projT[u, t] = sum_e Wk[u, e] * X[n, t, e]          (big matmul, [U, T] layout)
    th[u, t]      = tanh(K_projT[u, t] + q_proj[n, u])   (ACT, per-partition bias)
    scores[t]     = sum_u v[u] * th[u, t]                (PE, v as 1-col stationary)
    a[t]          = softmax(scores + mask[n])            (mask additive -1e30)
    ctx[e]        = sum_t a[t] * X[n, t, e]              (PE, aT cols as stationary)

Host precomputes q_proj = queries @ Wq.T (tiny), the additive mask from
`lengths`, X transposed per row ([E, T]) so the contraction dim lands on
SBUF partitions, plus small layout shuffles of Wk / v / q_proj.
"""

import numpy as np

import concourse.bass as bass
import concourse.mybir as mybir
import concourse.tile as tile
from concourse.bass_utils import run_bass_kernel_spmd

# Problem shape (hardcoded per contract; kernel.py must be self-contained).
N, T, D_ENC, D_DEC, U = 32, 2048, 1024, 1024, 1024
N_CORES = 8
R = N // N_CORES            # batch rows per core
P = 128                     # SBUF partitions
TC = 512                    # t-chunk = matmul moving free dim (fp32 max)
NTC = T // TC
ET = D_ENC // P             # e-tiles (contraction of the big matmul)
UT = U // P                 # u-tiles
TT = T // P                 # t-tiles (contraction of the context matmul)
EC = 512
NEC = D_ENC // EC

F32 = mybir.dt.float32
# PE matmul dtype. float32r = single-pass fp32 matmul (full rate at free
# dim >= 256); plain float32 = 2 half-speed passes (4x slower).
MM_DT = mybir.dt.float32r

AF = mybir.ActivationFunctionType
AX = mybir.AxisListType

MASK_NEG = np.float32(-1.0e30)

LAST_RESULTS = None         # BassKernelResults of the most recent run
_PROGRAM = None


def _mm(ap):
    return ap if ap.dtype == MM_DT else ap.bitcast(MM_DT)


def _legalize_waits(nc):
    """Several walrus instruction encodings (the self-loading fp32r matmul's
    S3_LW, Activation's S3D3_AC, ...) have a single sync-wait slot, but Tile
    sometimes emits 2+ waits on one instruction. Hoist the extra waits onto
    engine NoOps inserted just before the instruction — the engine's NX
    evaluates waits in program order, so gating is preserved. This covers
    HWDGE DMAs too: the issuing engine's sequencer writes the descriptor
    in program order, so a same-engine NoOp gates the transfer."""
    for f in nc.m.functions:
        for blk in f.blocks:
            insts = blk.instructions
            idx = 0
            while idx < len(insts):
                ins = insts[idx]
                if (
                    not isinstance(ins, mybir.InstCollectiveCompute)
                    and ins.engine is not None
                    and ins.sync_info is not None
                    and len(ins.sync_info.on_wait) > 1
                ):
                    waits = list(ins.sync_info.on_wait)
                    # one wait per NoOp — every ISA ctrl struct fits that
                    for w in waits[1:]:
                        nop = mybir.InstNoOp(
                            name=nc.get_next_instruction_name(), ins=[], outs=[]
                        )
                        nop.engine = ins.engine
                        nop.sync_info = mybir.SyncInfo(on_wait=[w], on_update=[])
                        insts.insert(idx, nop)
                        idx += 1
                    ins.sync_info = mybir.SyncInfo(
                        on_wait=[waits[0]], on_update=list(ins.sync_info.on_update)
                    )
                idx += 1


def build_program(legalize: bool = True) -> bass.Bass:
    nc = bass.Bass("TRN2")

    xt = nc.dram_tensor("xt", [R, D_ENC, T], F32, kind="ExternalInput").ap()
    xn = nc.dram_tensor("xn", [R, T, D_ENC], F32, kind="ExternalInput").ap()
    wkt = nc.dram_tensor("wkt", [P, UT, ET, P], F32, kind="ExternalInput").ap()
    vt = nc.dram_tensor("vt", [P, UT], F32, kind="ExternalInput").ap()
    qpt = nc.dram_tensor("qpt", [P, R * UT], F32, kind="ExternalInput").ap()
    mask = nc.dram_tensor("mask", [R, T], F32, kind="ExternalInput").ap()
    ctx_out = nc.dram_tensor("contexts", [R, D_ENC], F32, kind="ExternalOutput").ap()
    align_out = nc.dram_tensor("alignments", [R, T], F32, kind="ExternalOutput").ap()

    with tile.TileContext(nc) as tc:
        with (
            tc.tile_pool(name="const", bufs=1) as const_pool,
            tc.tile_pool(name="xtp", bufs=3) as xt_pool,
            tc.tile_pool(name="thp", bufs=4) as th_pool,
            tc.tile_pool(name="xnp", bufs=12) as xn_pool,
            tc.tile_pool(name="rowp", bufs=2) as row_pool,
            tc.tile_pool(name="psm", bufs=3, space="PSUM") as psum_m,
            tc.tile_pool(name="pss", bufs=2, space="PSUM") as psum_s,
            tc.tile_pool(name="psc", bufs=2, space="PSUM") as psum_c,
            tc.tile_pool(name="psj", bufs=1, space="PSUM") as psum_j,
            tc.tile_pool(name="dramp", bufs=2, space="DRAM") as dram_pool,
        ):
            # Replicated constants. wk is loaded per-ut-slice (512 KB each) so
            # the first matmul group only waits for 1/8 of Wk; the first xt
            # chunk's DMAs are emitted right after wk[ut=0] so compute starts
            # ~7 us in instead of waiting for all 6 MB of startup traffic.
            wk_sb = const_pool.tile([P, UT, ET, P], MM_DT, tag="wk")
            nc.sync.dma_start(wk_sb[:, 0], _mm(wkt[:, 0]))

            def load_xt_chunk(r, c):
                xt_sb = xt_pool.tile([P, ET, TC], MM_DT, tag="xt")
                for et in range(ET):
                    nc.sync.dma_start(
                        xt_sb[:, et],
                        _mm(xt[r, et * P:(et + 1) * P, c * TC:(c + 1) * TC]),
                    )
                return xt_sb

            xt_first = load_xt_chunk(0, 0)
            xt_second = load_xt_chunk(0, 1)
            for ut in range(1, UT):
                nc.sync.dma_start(wk_sb[:, ut], _mm(wkt[:, ut]))
            vt_sb = const_pool.tile([P, UT], MM_DT, tag="vt")
            nc.sync.dma_start(vt_sb[:], _mm(vt[:, :]))
            qpt_sb = const_pool.tile([P, R * UT], F32, tag="qpt")
            nc.sync.dma_start(qpt_sb[:], qpt[:, :])

            row_state = {}

            def phase_a_chunk(r, c, xt_sb, row):
                sc_ps = psum_s.tile([1, TC], F32, tag="sc")
                ths = []
                for ut in range(UT):
                    ps = psum_m.tile([P, TC], F32, tag="kproj")
                    for et in range(ET):
                        nc.tensor.matmul(
                            ps[:],
                            wk_sb[:, ut, et],
                            xt_sb[:, et],
                            start=(et == 0),
                            stop=(et == ET - 1),
                        )
                    th = th_pool.tile([P, TC], MM_DT, tag="th")
                    nc.scalar.activation(
                        th[:], ps[:], AF.Tanh,
                        bias=qpt_sb[:, r * UT + ut:r * UT + ut + 1],
                    )
                    ths.append(th)
                    # score matmul for ut-1: one main group behind, so the
                    # tanh it waits on is already finished (no PE stall).
                    if ut > 0:
                        nc.tensor.matmul(
                            sc_ps[:], vt_sb[:, ut - 1:ut], _mm(ths[ut - 1][:]),
                            start=(ut == 1), stop=False,
                        )
                nc.tensor.matmul(
                    sc_ps[:], vt_sb[:, UT - 1:UT], _mm(ths[UT - 1][:]),
                    start=False, stop=True,
                )
                nc.vector.tensor_add(
                    row["scores"][:, c * TC:(c + 1) * TC], sc_ps[:],
                    row["mask"][:, c * TC:(c + 1) * TC],
                )
                # per-chunk max, so the end-of-row reduction is tiny and the
                # PE gap before the context matmuls stays under the HAM
                # re-throttle window.
                nc.vector.reduce_max(
                    row["mx4"][:, c:c + 1],
                    row["scores"][:, c * TC:(c + 1) * TC], axis=AX.X,
                )

            def keepalive(dep_ap):
                # tiny real matmul that reads the given (fp32r) tile: threads
                # a PE instruction through the softmax chain so the HAM
                # activity monitor never sees an idle window and the context
                # matmuls that follow run at full clock.
                jp = psum_j.tile([1, TC], F32, tag="junk", name="junk_ps")
                n = dep_ap.shape[-1]
                nc.tensor.matmul(
                    jp[:, 0:n], dep_ap[0:1, 0:1], dep_ap[0:1, :],
                    start=True, stop=True,
                )

            def softmax_row(r, row):
                mxn = row_pool.tile([1, 1], F32, tag="mxn")
                nc.vector.reduce_max(mxn[:], row["mx4"][:], axis=AX.X, negate=True)
                keepalive(row["mx4"][:])
                exp_sb = row_pool.tile([1, T], MM_DT, tag="exp")
                zsum = row_pool.tile([1, 1], F32, tag="z")
                nc.scalar.activation(
                    exp_sb[:], row["scores"][:], AF.Exp, bias=mxn[:],
                    accum_out=zsum[:],
                )
                keepalive(exp_sb[:, 0:TC])
                rz = row_pool.tile([1, 1], F32, tag="rz")
                nc.vector.reciprocal(rz[:], zsum[:])
                keepalive(exp_sb[:, TC:2 * TC])
                # aT via DRAM bounce (unnormalized; 1/Z folded into outputs)
                bounce = dram_pool.tile([1, T], MM_DT, tag="bounce")
                nc.sync.dma_start(bounce[:], exp_sb[:])
                at_sb = row_pool.tile([P, TT], MM_DT, tag="at")
                nc.sync.dma_start(
                    at_sb[:], bounce[0].rearrange("(j p) -> p j", p=P)
                )
                keepalive(exp_sb[:, 2 * TC:3 * TC])
                align_sb = row["scores"]
                nc.scalar.activation(align_sb[:], exp_sb[:], AF.Copy, scale=rz[:])
                nc.sync.dma_start(align_out[r:r + 1, :], align_sb[:].bitcast(F32))
                keepalive(align_sb[:, 0:TC])
                row["at"], row["rz"] = at_sb, rz

            def phase_b_load(r, row):
                tiles = []
                for tt in range(TT):
                    xn_sb = xn_pool.tile([P, D_ENC], MM_DT, tag="xn")
                    nc.sync.dma_start(xn_sb[:], _mm(xn[r, tt * P:(tt + 1) * P, :]))
                    tiles.append(xn_sb)
                row["xn"] = tiles

            def phase_b_row(r, row):
                # ctx[e] = (1/Z) * sum_t exp[t] * X[t, e]
                ct_ps0 = psum_c.tile([1, EC], F32, tag="ctx")
                ct_ps1 = psum_c.tile([1, EC], F32, tag="ctx")
                at_sb, rz = row["at"], row["rz"]
                for tt in range(TT):
                    xn_sb = row["xn"][tt]
                    nc.tensor.matmul(
                        ct_ps0[:], at_sb[:, tt:tt + 1], xn_sb[:, 0:EC],
                        start=(tt == 0), stop=(tt == TT - 1),
                    )
                    nc.tensor.matmul(
                        ct_ps1[:], at_sb[:, tt:tt + 1], xn_sb[:, EC:2 * EC],
                        start=(tt == 0), stop=(tt == TT - 1),
                    )
                ctx_sb = row_pool.tile([1, D_ENC], F32, tag="ctx_sb")
                nc.scalar.activation(ctx_sb[:, 0:EC], ct_ps0[:], AF.Copy, scale=rz[:])
                nc.scalar.activation(ctx_sb[:, EC:], ct_ps1[:], AF.Copy, scale=rz[:])
                nc.sync.dma_start(ctx_out[r:r + 1, :], ctx_sb[:])

            for r in range(R):
                mask_sb = row_pool.tile([1, T], F32, tag="mask", bufs=1)
                nc.sync.dma_start(mask_sb[:], mask[r:r + 1, :])
                row = {
                    "mask": mask_sb,
                    "scores": row_pool.tile([1, T], MM_DT, tag="scores", name="scores_sb"),
                    "mx4": row_pool.tile([1, NTC], MM_DT, tag="mx4", name="mx4_sb"),
                }
                row_state[r] = row
                for c in range(NTC):
                    if r == 0 and c == 0:
                        xt_sb = xt_first
                    elif r == 0 and c == 1:
                        xt_sb = xt_second
                    else:
                        xt_sb = load_xt_chunk(r, c)
                    if c == 3:
                        phase_b_load(r, row)
                    phase_a_chunk(r, c, xt_sb, row)
                    # previous row's context matmuls slot in here, one chunk
                    # deep into this row, so PE never stalls on its softmax.
                    if r > 0 and c == 1:
                        phase_b_row(r - 1, row_state[r - 1])
                softmax_row(r, row)
            phase_b_row(R - 1, row_state[R - 1])

    if legalize:
        _legalize_waits(nc)
    return nc


def _get_program() -> bass.Bass:
    global _PROGRAM
    if _PROGRAM is None:
        _PROGRAM = build_program()
    return _PROGRAM


def make_in_maps(queries, encoder_output, lengths, v, Wq, Wk):
    """Host-side marshalling: shard batch across cores + layout shuffles."""
    queries = np.ascontiguousarray(np.asarray(queries), dtype=np.float32)
    encoder_output = np.ascontiguousarray(np.asarray(encoder_output), dtype=np.float32)
    lengths = np.asarray(lengths).astype(np.int64)
    v = np.asarray(v, dtype=np.float32)
    Wq = np.asarray(Wq, dtype=np.float32)
    Wk = np.asarray(Wk, dtype=np.float32)

    qp = queries[:, 0, :] @ Wq.T                                   # [N, U]
    xt_full = np.ascontiguousarray(encoder_output.transpose(0, 2, 1))  # [N, E, T]
    # wkt[p, ut, et, j] = Wk[ut*128+j, et*128+p]
    wkt = np.ascontiguousarray(Wk.reshape(UT, P, ET, P).transpose(3, 0, 2, 1))
    vt = np.ascontiguousarray(v.reshape(UT, P).T)                  # [P, UT]
    mask = np.where(
        np.arange(T)[None, :] >= lengths[:, None], MASK_NEG, np.float32(0.0)
    ).astype(np.float32)                                           # [N, T]

    in_maps = []
    for i in range(N_CORES):
        sl = slice(i * R, (i + 1) * R)
        qpt = np.ascontiguousarray(
            qp[sl].reshape(R, UT, P).transpose(2, 0, 1).reshape(P, R * UT)
        )
        in_maps.append({
            "xt": xt_full[sl],
            "xn": encoder_output[sl],
            "wkt": wkt,
            "vt": vt,
            "qpt": qpt,
            "mask": np.ascontiguousarray(mask[sl]),
        })
    return in_maps


def kernel(queries, encoder_output, lengths, v, Wq, Wk, _trace=False):
    global LAST_RESULTS
    in_maps = make_in_maps(queries, encoder_output, lengths, v, Wq, Wk)
    nc = _get_program()
    res = run_bass_kernel_spmd(
        nc, in_maps, core_ids=list(range(N_CORES)), trace=_trace
    )
    LAST_RESULTS = res
    contexts = np.concatenate(
        [res.results[i]["contexts"] for i in range(N_CORES)], axis=0
    )
    alignments = np.concatenate(
        [res.results[i]["alignments"] for i in range(N_CORES)], axis=0
    )
    return contexts, alignments


# revision 22
# speedup vs baseline: 1.1717x; 1.0139x over previous
"""Bahdanau attention on TRN2 — data-parallel over batch across 8 NeuronCores.

Math per batch row n (shapes: T=2048 encoder steps, E=U=1024):
    K_projT[u, t] = sum_e Wk[u, e] * X[n, t, e]          (big matmul, [U, T] layout)
    th[u, t]      = tanh(K_projT[u, t] + q_proj[n, u])   (ACT, per-partition bias)
    scores[t]     = sum_u v[u] * th[u, t]                (PE, v as 1-col stationary)
    a[t]          = softmax(scores + mask[n])            (mask additive -1e30)
    ctx[e]        = sum_t a[t] * X[n, t, e]              (PE, aT cols as stationary)

Host precomputes q_proj = queries @ Wq.T (tiny), the additive mask from
`lengths`, X transposed per row ([E, T]) so the contraction dim lands on
SBUF partitions, plus small layout shuffles of Wk / v / q_proj.
"""

import ml_dtypes
import numpy as np

import concourse.bass as bass
import concourse.mybir as mybir
import concourse.tile as tile
from concourse.bass_utils import run_bass_kernel_spmd

# Problem shape (hardcoded per contract; kernel.py must be self-contained).
N, T, D_ENC, D_DEC, U = 32, 2048, 1024, 1024, 1024
N_CORES = 8
R = N // N_CORES            # batch rows per core
P = 128                     # SBUF partitions
TC = 512                    # t-chunk = matmul moving free dim (fp32 max)
NTC = T // TC
ET = D_ENC // P             # e-tiles (contraction of the big matmul)
UT = U // P                 # u-tiles
TT = T // P                 # t-tiles (contraction of the context matmul)
EC = 512
NEC = D_ENC // EC

F32 = mybir.dt.float32
# PE matmul dtype. float32r = single-pass fp32 matmul (full rate at free
# dim >= 256); plain float32 = 2 half-speed passes (4x slower).
MM_DT = mybir.dt.float32r

AF = mybir.ActivationFunctionType
AX = mybir.AxisListType

MASK_NEG = np.float32(-1.0e30)

LAST_RESULTS = None         # BassKernelResults of the most recent run
_PROGRAM = None


def _mm(ap):
    return ap if ap.dtype == MM_DT else ap.bitcast(MM_DT)


def _legalize_waits(nc):
    """Several walrus instruction encodings (the self-loading fp32r matmul's
    S3_LW, Activation's S3D3_AC, ...) have a single sync-wait slot, but Tile
    sometimes emits 2+ waits on one instruction. Hoist the extra waits onto
    engine NoOps inserted just before the instruction — the engine's NX
    evaluates waits in program order, so gating is preserved. This covers
    HWDGE DMAs too: the issuing engine's sequencer writes the descriptor
    in program order, so a same-engine NoOp gates the transfer."""
    for f in nc.m.functions:
        for blk in f.blocks:
            insts = blk.instructions
            idx = 0
            while idx < len(insts):
                ins = insts[idx]
                if (
                    not isinstance(ins, mybir.InstCollectiveCompute)
                    and ins.engine is not None
                    and ins.sync_info is not None
                    and len(ins.sync_info.on_wait) > 1
                ):
                    waits = list(ins.sync_info.on_wait)
                    # one wait per NoOp — every ISA ctrl struct fits that
                    for w in waits[1:]:
                        nop = mybir.InstNoOp(
                            name=nc.get_next_instruction_name(), ins=[], outs=[]
                        )
                        nop.engine = ins.engine
                        nop.sync_info = mybir.SyncInfo(on_wait=[w], on_update=[])
                        insts.insert(idx, nop)
                        idx += 1
                    ins.sync_info = mybir.SyncInfo(
                        on_wait=[waits[0]], on_update=list(ins.sync_info.on_update)
                    )
                idx += 1


def build_program(legalize: bool = True) -> bass.Bass:
    nc = bass.Bass("TRN2")

    xt = nc.dram_tensor("xt", [R, D_ENC, T], F32, kind="ExternalInput").ap()
    xn = nc.dram_tensor("xn", [R, T, D_ENC], F32, kind="ExternalInput").ap()
    wkt = nc.dram_tensor("wkt", [P, UT, ET, P], F32, kind="ExternalInput").ap()
    vt = nc.dram_tensor("vt", [P, UT], F32, kind="ExternalInput").ap()
    qpt = nc.dram_tensor("qpt", [P, R * UT], F32, kind="ExternalInput").ap()
    mask = nc.dram_tensor("mask", [1, R * T], mybir.dt.bfloat16, kind="ExternalInput").ap()
    ctx_out = nc.dram_tensor("contexts", [R, D_ENC], F32, kind="ExternalOutput").ap()
    align_out = nc.dram_tensor("alignments", [R, T], F32, kind="ExternalOutput").ap()

    with tile.TileContext(nc) as tc:
        with (
            tc.tile_pool(name="const", bufs=1) as const_pool,
            tc.tile_pool(name="xtp", bufs=3) as xt_pool,
            tc.tile_pool(name="thp", bufs=4) as th_pool,
            tc.tile_pool(name="xnp", bufs=12) as xn_pool,
            tc.tile_pool(name="rowp", bufs=2) as row_pool,
            tc.tile_pool(name="psm", bufs=3, space="PSUM") as psum_m,
            tc.tile_pool(name="pss", bufs=2, space="PSUM") as psum_s,
            tc.tile_pool(name="psc", bufs=2, space="PSUM") as psum_c,
            tc.tile_pool(name="psj", bufs=1, space="PSUM") as psum_j,
            tc.tile_pool(name="dramp", bufs=2, space="DRAM") as dram_pool,
        ):
            # Replicated constants + startup interleave. All HWDGE DMAs
            # execute as one serial stream in issue order (each transfer
            # alone saturates ~430 GB/s), so ordering is everything: tiny
            # tensors first (they gate the tanh/score chain), then Wk
            # u-slices interleaved with the first xt chunks to match the
            # PE's consumption order.
            vt_sb = const_pool.tile([P, UT], MM_DT, tag="vt")
            nc.sync.dma_start(vt_sb[:], _mm(vt[:, :]))
            qpt_sb = const_pool.tile([P, R * UT], F32, tag="qpt")
            nc.sync.dma_start(qpt_sb[:], qpt[:, :])
            mask_sb = const_pool.tile([1, R * T], mybir.dt.bfloat16, tag="mask")
            nc.sync.dma_start(mask_sb[:], mask[:, :])

            wk_sb = const_pool.tile([P, UT, ET, P], MM_DT, tag="wk")

            def load_wk(ut):
                nc.sync.dma_start(wk_sb[:, ut], _mm(wkt[:, ut]))

            def load_xt_part(xt_sb, r, c, ets):
                for et in ets:
                    nc.sync.dma_start(
                        xt_sb[:, et],
                        _mm(xt[r, et * P:(et + 1) * P, c * TC:(c + 1) * TC]),
                    )

            def load_xt_chunk(r, c):
                xt_sb = xt_pool.tile([P, ET, TC], MM_DT, tag="xt")
                load_xt_part(xt_sb, r, c, range(ET))
                return xt_sb

            load_wk(0)
            xt_first = xt_pool.tile([P, ET, TC], MM_DT, tag="xt", name="xt_sb")
            load_xt_part(xt_first, 0, 0, range(ET))
            load_wk(1)
            load_wk(2)
            load_wk(3)
            xt_second = xt_pool.tile([P, ET, TC], MM_DT, tag="xt", name="xt_sb")
            load_xt_part(xt_second, 0, 1, range(4))
            load_wk(4)
            load_wk(5)
            load_xt_part(xt_second, 0, 1, range(4, ET))
            load_wk(6)
            load_wk(7)

            row_state = {}

            def phase_a_chunk(r, c, xt_sb, row):
                sc_ps = psum_s.tile([1, TC], F32, tag="sc")
                ths = []
                for ut in range(UT):
                    ps = psum_m.tile([P, TC], F32, tag="kproj")
                    for et in range(ET):
                        nc.tensor.matmul(
                            ps[:],
                            wk_sb[:, ut, et],
                            xt_sb[:, et],
                            start=(et == 0),
                            stop=(et == ET - 1),
                        )
                    th = th_pool.tile([P, TC], MM_DT, tag="th")
                    nc.scalar.activation(
                        th[:], ps[:], AF.Tanh,
                        bias=qpt_sb[:, r * UT + ut:r * UT + ut + 1],
                    )
                    ths.append(th)
                    # score matmul for ut-1: one main group behind, so the
                    # tanh it waits on is already finished (no PE stall).
                    if ut > 0:
                        nc.tensor.matmul(
                            sc_ps[:], vt_sb[:, ut - 1:ut], _mm(ths[ut - 1][:]),
                            start=(ut == 1), stop=False,
                        )
                nc.tensor.matmul(
                    sc_ps[:], vt_sb[:, UT - 1:UT], _mm(ths[UT - 1][:]),
                    start=False, stop=True,
                )
                nc.vector.tensor_add(
                    row["scores"][:, c * TC:(c + 1) * TC], sc_ps[:],
                    row["mask"][:, c * TC:(c + 1) * TC],
                )
                # per-chunk max, so the end-of-row reduction is tiny and the
                # PE gap before the context matmuls stays under the HAM
                # re-throttle window.
                nc.vector.reduce_max(
                    row["mx4"][:, c:c + 1],
                    row["scores"][:, c * TC:(c + 1) * TC], axis=AX.X,
                )

            def keepalive(dep_ap):
                # tiny real matmul that reads the given (fp32r) tile: threads
                # a PE instruction through the softmax chain so the HAM
                # activity monitor never sees an idle window and the context
                # matmuls that follow run at full clock.
                jp = psum_j.tile([1, TC], F32, tag="junk", name="junk_ps")
                n = dep_ap.shape[-1]
                nc.tensor.matmul(
                    jp[:, 0:n], dep_ap[0:1, 0:1], dep_ap[0:1, :],
                    start=True, stop=True,
                )

            def softmax_row(r, row):
                mxn = row_pool.tile([1, 1], F32, tag="mxn")
                nc.vector.reduce_max(mxn[:], row["mx4"][:], axis=AX.X, negate=True)
                keepalive(row["mx4"][:])
                exp_sb = row_pool.tile([1, T], MM_DT, tag="exp")
                zsum = row_pool.tile([1, 1], F32, tag="z")
                nc.scalar.activation(
                    exp_sb[:], row["scores"][:], AF.Exp, bias=mxn[:],
                    accum_out=zsum[:],
                )
                keepalive(exp_sb[:, 0:TC])
                rz = row_pool.tile([1, 1], F32, tag="rz")
                nc.vector.reciprocal(rz[:], zsum[:])
                keepalive(exp_sb[:, TC:2 * TC])
                # aT via DRAM bounce (SBUF->SBUF partition-scatter DMA
                # returns garbage on HW); unnormalized — 1/Z is folded into
                # the final outputs.
                bounce = dram_pool.tile([1, T], MM_DT, tag="bounce")
                nc.sync.dma_start(bounce[:], exp_sb[:])
                at_sb = row_pool.tile([P, TT], MM_DT, tag="at")
                nc.sync.dma_start(
                    at_sb[:], bounce[0].rearrange("(j p) -> p j", p=P)
                )
                keepalive(exp_sb[:, 2 * TC:3 * TC])
                align_sb = row["scores"]
                nc.scalar.activation(align_sb[:], exp_sb[:], AF.Copy, scale=rz[:])
                nc.sync.dma_start(align_out[r:r + 1, :], align_sb[:])
                row["at"], row["rz"] = at_sb, rz

            def phase_b_load(r, row):
                tiles = []
                for tt in range(TT):
                    xn_sb = xn_pool.tile([P, D_ENC], MM_DT, tag="xn")
                    nc.sync.dma_start(xn_sb[:], _mm(xn[r, tt * P:(tt + 1) * P, :]))
                    tiles.append(xn_sb)
                row["xn"] = tiles

            def phase_b_row(r, row):
                # ctx[e] = (1/Z) * sum_t exp[t] * X[t, e]
                ct_ps0 = psum_c.tile([1, EC], F32, tag="ctx")
                ct_ps1 = psum_c.tile([1, EC], F32, tag="ctx")
                at_sb, rz = row["at"], row["rz"]
                for tt in range(TT):
                    xn_sb = row["xn"][tt]
                    nc.tensor.matmul(
                        ct_ps0[:], at_sb[:, tt:tt + 1], xn_sb[:, 0:EC],
                        start=(tt == 0), stop=(tt == TT - 1),
                    )
                    nc.tensor.matmul(
                        ct_ps1[:], at_sb[:, tt:tt + 1], xn_sb[:, EC:2 * EC],
                        start=(tt == 0), stop=(tt == TT - 1),
                    )
                ctx_sb = row_pool.tile([1, D_ENC], F32, tag="ctx_sb", bufs=1)
                nc.scalar.activation(ctx_sb[:, 0:EC], ct_ps0[:], AF.Copy, scale=rz[:])
                nc.scalar.activation(ctx_sb[:, EC:], ct_ps1[:], AF.Copy, scale=rz[:])
                nc.sync.dma_start(ctx_out[r:r + 1, :], ctx_sb[:])

            for r in range(R):
                row = {
                    "mask": mask_sb[:, r * T:(r + 1) * T],
                    "scores": row_pool.tile([1, T], F32, tag="scores", name="scores_sb"),
                    "mx4": row_pool.tile([1, NTC], MM_DT, tag="mx4", name="mx4_sb"),
                }
                row_state[r] = row
                for c in range(NTC):
                    if r == 0 and c == 0:
                        xt_sb = xt_first
                    elif r == 0 and c == 1:
                        xt_sb = xt_second
                    else:
                        xt_sb = load_xt_chunk(r, c)
                    if c == 3:
                        phase_b_load(r, row)
                    phase_a_chunk(r, c, xt_sb, row)
                    # previous row's context matmuls slot in here, one chunk
                    # deep into this row, so PE never stalls on its softmax.
                    if r > 0 and c == 1:
                        phase_b_row(r - 1, row_state[r - 1])
                softmax_row(r, row)
            phase_b_row(R - 1, row_state[R - 1])

    if legalize:
        _legalize_waits(nc)
    return nc


def _get_program() -> bass.Bass:
    global _PROGRAM
    if _PROGRAM is None:
        _PROGRAM = build_program()
    return _PROGRAM


def make_in_maps(queries, encoder_output, lengths, v, Wq, Wk):
    """Host-side marshalling: shard batch across cores + layout shuffles."""
    queries = np.ascontiguousarray(np.asarray(queries), dtype=np.float32)
    encoder_output = np.ascontiguousarray(np.asarray(encoder_output), dtype=np.float32)
    lengths = np.asarray(lengths).astype(np.int64)
    v = np.asarray(v, dtype=np.float32)
    Wq = np.asarray(Wq, dtype=np.float32)
    Wk = np.asarray(Wk, dtype=np.float32)

    qp = queries[:, 0, :] @ Wq.T                                   # [N, U]
    xt_full = np.ascontiguousarray(encoder_output.transpose(0, 2, 1))  # [N, E, T]
    # wkt[p, ut, et, j] = Wk[ut*128+j, et*128+p]
    wkt = np.ascontiguousarray(Wk.reshape(UT, P, ET, P).transpose(3, 0, 2, 1))
    vt = np.ascontiguousarray(v.reshape(UT, P).T)                  # [P, UT]
    mask = np.where(
        np.arange(T)[None, :] >= lengths[:, None], MASK_NEG, np.float32(0.0)
    ).astype(np.float32)                                           # [N, T]

    in_maps = []
    for i in range(N_CORES):
        sl = slice(i * R, (i + 1) * R)
        qpt = np.ascontiguousarray(
            qp[sl].reshape(R, UT, P).transpose(2, 0, 1).reshape(P, R * UT)
        )
        in_maps.append({
            "xt": xt_full[sl],
            "xn": encoder_output[sl],
            "wkt": wkt,
            "vt": vt,
            "qpt": qpt,
            "mask": np.ascontiguousarray(mask[sl].reshape(1, R * T)).astype(ml_dtypes.bfloat16),
        })
    return in_maps


def kernel(queries, encoder_output, lengths, v, Wq, Wk, _trace=False):
    global LAST_RESULTS
    in_maps = make_in_maps(queries, encoder_output, lengths, v, Wq, Wk)
    nc = _get_program()
    res = run_bass_kernel_spmd(
        nc, in_maps, core_ids=list(range(N_CORES)), trace=_trace
    )
    LAST_RESULTS = res
    contexts = np.concatenate(
        [res.results[i]["contexts"] for i in range(N_CORES)], axis=0
    )
    alignments = np.concatenate(
        [res.results[i]["alignments"] for i in range(N_CORES)], axis=0
    )
    return contexts, alignments


# revision 24
# speedup vs baseline: 1.1854x; 1.0117x over previous
"""Bahdanau attention on TRN2 — data-parallel over batch across 8 NeuronCores.

Math per batch row n (shapes: T=2048 encoder steps, E=U=1024):
    K_projT[u, t] = sum_e Wk[u, e] * X[n, t, e]          (big matmul, [U, T] layout)
    th[u, t]      = tanh(K_projT[u, t] + q_proj[n, u])   (ACT, per-partition bias)
    scores[t]     = sum_u v[u] * th[u, t]                (PE, v as 1-col stationary)
    a[t]          = softmax(scores + mask[n])            (mask additive -1e30)
    ctx[e]        = sum_t a[t] * X[n, t, e]              (PE, aT cols as stationary)

Host precomputes q_proj = queries @ Wq.T (tiny), the additive mask from
`lengths`, X transposed per row ([E, T]) so the contraction dim lands on
SBUF partitions, plus small layout shuffles of Wk / v / q_proj.
"""

import ml_dtypes
import numpy as np

import concourse.bass as bass
import concourse.mybir as mybir
import concourse.tile as tile
from concourse.bass_utils import run_bass_kernel_spmd

# Problem shape (hardcoded per contract; kernel.py must be self-contained).
N, T, D_ENC, D_DEC, U = 32, 2048, 1024, 1024, 1024
N_CORES = 8
R = N // N_CORES            # batch rows per core
P = 128                     # SBUF partitions
TC = 512                    # t-chunk = matmul moving free dim (fp32 max)
NTC = T // TC
ET = D_ENC // P             # e-tiles (contraction of the big matmul)
UT = U // P                 # u-tiles
TT = T // P                 # t-tiles (contraction of the context matmul)
EC = 512
NEC = D_ENC // EC

F32 = mybir.dt.float32
# PE matmul dtype. float32r = single-pass fp32 matmul (full rate at free
# dim >= 256); plain float32 = 2 half-speed passes (4x slower).
MM_DT = mybir.dt.float32r

AF = mybir.ActivationFunctionType
AX = mybir.AxisListType

MASK_NEG = np.float32(-1.0e30)

LAST_RESULTS = None         # BassKernelResults of the most recent run
_PROGRAM = None


def _mm(ap):
    return ap if ap.dtype == MM_DT else ap.bitcast(MM_DT)


def _legalize_waits(nc):
    """Several walrus instruction encodings (the self-loading fp32r matmul's
    S3_LW, Activation's S3D3_AC, ...) have a single sync-wait slot, but Tile
    sometimes emits 2+ waits on one instruction. Hoist the extra waits onto
    engine NoOps inserted just before the instruction — the engine's NX
    evaluates waits in program order, so gating is preserved. This covers
    HWDGE DMAs too: the issuing engine's sequencer writes the descriptor
    in program order, so a same-engine NoOp gates the transfer."""
    for f in nc.m.functions:
        for blk in f.blocks:
            insts = blk.instructions
            idx = 0
            while idx < len(insts):
                ins = insts[idx]
                if (
                    not isinstance(ins, mybir.InstCollectiveCompute)
                    and ins.engine is not None
                    and ins.sync_info is not None
                    and len(ins.sync_info.on_wait) > 1
                ):
                    waits = list(ins.sync_info.on_wait)
                    # one wait per NoOp — every ISA ctrl struct fits that
                    for w in waits[1:]:
                        nop = mybir.InstNoOp(
                            name=nc.get_next_instruction_name(), ins=[], outs=[]
                        )
                        nop.engine = ins.engine
                        nop.sync_info = mybir.SyncInfo(on_wait=[w], on_update=[])
                        insts.insert(idx, nop)
                        idx += 1
                    ins.sync_info = mybir.SyncInfo(
                        on_wait=[waits[0]], on_update=list(ins.sync_info.on_update)
                    )
                idx += 1


def build_program(legalize: bool = True) -> bass.Bass:
    nc = bass.Bass("TRN2")

    xt = nc.dram_tensor("xt", [R, D_ENC, T], F32, kind="ExternalInput").ap()
    xn = nc.dram_tensor("xn", [R, T, D_ENC], F32, kind="ExternalInput").ap()
    wkt = nc.dram_tensor("wkt", [P, UT, ET, P], F32, kind="ExternalInput").ap()
    vq = nc.dram_tensor("vq", [P, UT + R * UT], F32, kind="ExternalInput").ap()
    mask = nc.dram_tensor("mask", [1, R * T], mybir.dt.bfloat16, kind="ExternalInput").ap()
    ctx_out = nc.dram_tensor("contexts", [R, D_ENC], F32, kind="ExternalOutput").ap()
    align_out = nc.dram_tensor("alignments", [R, T], F32, kind="ExternalOutput").ap()

    with tile.TileContext(nc) as tc:
        with (
            tc.tile_pool(name="const", bufs=1) as const_pool,
            tc.tile_pool(name="xtp", bufs=3) as xt_pool,
            tc.tile_pool(name="thp", bufs=4) as th_pool,
            tc.tile_pool(name="xnp", bufs=12) as xn_pool,
            tc.tile_pool(name="rowp", bufs=2) as row_pool,
            tc.tile_pool(name="psm", bufs=3, space="PSUM") as psum_m,
            tc.tile_pool(name="pss", bufs=2, space="PSUM") as psum_s,
            tc.tile_pool(name="psc", bufs=2, space="PSUM") as psum_c,
            tc.tile_pool(name="psj", bufs=1, space="PSUM") as psum_j,
            tc.tile_pool(name="dramp", bufs=2, space="DRAM") as dram_pool,
        ):
            # Replicated constants + startup interleave. All HWDGE DMAs
            # execute as one serial stream in issue order (each transfer
            # alone saturates ~430 GB/s), so ordering is everything: tiny
            # tensors first (they gate the tanh/score chain), then Wk
            # u-slices interleaved with the first xt chunks to match the
            # PE's consumption order.
            vq_sb = const_pool.tile([P, UT + R * UT], MM_DT, tag="vq")
            nc.sync.dma_start(vq_sb[:], _mm(vq[:, :]))
            vt_sb = vq_sb[:, 0:UT]
            # same bits, fp32 view — the DMA copies bits, nothing is rounded
            qpt_sb = vq_sb[:, UT:UT + R * UT].bitcast(F32)
            mask_sb = const_pool.tile([1, R * T], mybir.dt.bfloat16, tag="mask")
            nc.sync.dma_start(mask_sb[:], mask[:, :])

            wk_sb = const_pool.tile([P, UT, ET, P], MM_DT, tag="wk")

            def load_wk(ut):
                nc.sync.dma_start(wk_sb[:, ut], _mm(wkt[:, ut]))

            def load_xt_part(xt_sb, r, c, ets):
                for et in ets:
                    nc.sync.dma_start(
                        xt_sb[:, et],
                        _mm(xt[r, et * P:(et + 1) * P, c * TC:(c + 1) * TC]),
                    )

            def load_xt_chunk(r, c):
                xt_sb = xt_pool.tile([P, ET, TC], MM_DT, tag="xt")
                load_xt_part(xt_sb, r, c, range(ET))
                return xt_sb

            load_wk(0)
            xt_first = xt_pool.tile([P, ET, TC], MM_DT, tag="xt", name="xt_sb")
            load_xt_part(xt_first, 0, 0, range(ET))
            load_wk(1)
            load_wk(2)
            load_wk(3)
            xt_second = xt_pool.tile([P, ET, TC], MM_DT, tag="xt", name="xt_sb")
            load_xt_part(xt_second, 0, 1, range(4))
            load_wk(4)
            load_wk(5)
            load_xt_part(xt_second, 0, 1, range(4, ET))
            load_wk(6)
            load_wk(7)

            row_state = {}

            def phase_a_chunk(r, c, xt_sb, row):
                sc_ps = psum_s.tile([1, TC], F32, tag="sc")
                ths = []
                for ut in range(UT):
                    ps = psum_m.tile([P, TC], F32, tag="kproj")
                    for et in range(ET):
                        nc.tensor.matmul(
                            ps[:],
                            wk_sb[:, ut, et],
                            xt_sb[:, et],
                            start=(et == 0),
                            stop=(et == ET - 1),
                        )
                    th = th_pool.tile([P, TC], MM_DT, tag="th")
                    nc.scalar.activation(
                        th[:], ps[:], AF.Tanh,
                        bias=qpt_sb[:, r * UT + ut:r * UT + ut + 1],
                    )
                    ths.append(th)
                    # score matmul for ut-1: one main group behind, so the
                    # tanh it waits on is already finished (no PE stall).
                    if ut > 0:
                        nc.tensor.matmul(
                            sc_ps[:], vt_sb[:, ut - 1:ut], _mm(ths[ut - 1][:]),
                            start=(ut == 1), stop=False,
                        )
                nc.tensor.matmul(
                    sc_ps[:], vt_sb[:, UT - 1:UT], _mm(ths[UT - 1][:]),
                    start=False, stop=True,
                )
                nc.vector.tensor_add(
                    row["scores"][:, c * TC:(c + 1) * TC], sc_ps[:],
                    row["mask"][:, c * TC:(c + 1) * TC],
                )
                # per-chunk max, so the end-of-row reduction is tiny and the
                # PE gap before the context matmuls stays under the HAM
                # re-throttle window.
                nc.vector.reduce_max(
                    row["mx4"][:, c:c + 1],
                    row["scores"][:, c * TC:(c + 1) * TC], axis=AX.X,
                )

            def keepalive(dep_ap):
                # tiny real matmul that reads the given (fp32r) tile: threads
                # a PE instruction through the softmax chain so the HAM
                # activity monitor never sees an idle window and the context
                # matmuls that follow run at full clock.
                jp = psum_j.tile([1, TC], F32, tag="junk", name="junk_ps")
                n = dep_ap.shape[-1]
                nc.tensor.matmul(
                    jp[:, 0:n], dep_ap[0:1, 0:1], dep_ap[0:1, :],
                    start=True, stop=True,
                )

            def softmax_row(r, row):
                mxn = row_pool.tile([1, 1], F32, tag="mxn")
                nc.vector.reduce_max(mxn[:], row["mx4"][:], axis=AX.X, negate=True)
                keepalive(row["mx4"][:])
                exp_sb = row_pool.tile([1, T], MM_DT, tag="exp")
                z4 = row_pool.tile([1, NTC], F32, tag="z4")
                bounce = dram_pool.tile([1, T], MM_DT, tag="bounce")
                at_sb = row_pool.tile([P, TT], MM_DT, tag="at")
                # exp -> DRAM bounce -> partition-scatter, pipelined per
                # 512-chunk: the first context matmuls only need at[:, 0:4],
                # so the PE resumes ~3 us earlier, and each chunk threads a
                # keepalive matmul so the PE clock never re-throttles.
                for c in range(NTC):
                    cs = slice(c * TC, (c + 1) * TC)
                    nc.scalar.activation(
                        exp_sb[:, cs], row["scores"][:, cs], AF.Exp,
                        bias=mxn[:], accum_out=z4[:, c:c + 1],
                    )
                    nc.sync.dma_start(bounce[:, cs], exp_sb[:, cs])
                    nc.sync.dma_start(
                        at_sb[:, c * (TC // P):(c + 1) * (TC // P)],
                        bounce[0, cs].rearrange("(j p) -> p j", p=P),
                    )
                    keepalive(exp_sb[:, cs])
                zsum = row_pool.tile([1, 1], F32, tag="z")
                nc.vector.reduce_sum(zsum[:], z4[:], axis=AX.X)
                rz = row_pool.tile([1, 1], F32, tag="rz")
                nc.vector.reciprocal(rz[:], zsum[:])
                align_sb = row["scores"]
                nc.scalar.activation(align_sb[:], exp_sb[:], AF.Copy, scale=rz[:])
                nc.sync.dma_start(align_out[r:r + 1, :], align_sb[:])
                row["at"], row["rz"] = at_sb, rz

            def phase_b_load(r, row):
                tiles = []
                for tt in range(TT):
                    xn_sb = xn_pool.tile([P, D_ENC], MM_DT, tag="xn")
                    nc.sync.dma_start(xn_sb[:], _mm(xn[r, tt * P:(tt + 1) * P, :]))
                    tiles.append(xn_sb)
                row["xn"] = tiles

            def phase_b_row(r, row):
                # ctx[e] = (1/Z) * sum_t exp[t] * X[t, e]
                ct_ps0 = psum_c.tile([1, EC], F32, tag="ctx")
                ct_ps1 = psum_c.tile([1, EC], F32, tag="ctx")
                at_sb, rz = row["at"], row["rz"]
                for tt in range(TT):
                    xn_sb = row["xn"][tt]
                    nc.tensor.matmul(
                        ct_ps0[:], at_sb[:, tt:tt + 1], xn_sb[:, 0:EC],
                        start=(tt == 0), stop=(tt == TT - 1),
                    )
                    nc.tensor.matmul(
                        ct_ps1[:], at_sb[:, tt:tt + 1], xn_sb[:, EC:2 * EC],
                        start=(tt == 0), stop=(tt == TT - 1),
                    )
                ctx_sb = row_pool.tile([1, D_ENC], F32, tag="ctx_sb", bufs=1)
                nc.scalar.activation(ctx_sb[:, 0:EC], ct_ps0[:], AF.Copy, scale=rz[:])
                nc.scalar.activation(ctx_sb[:, EC:], ct_ps1[:], AF.Copy, scale=rz[:])
                nc.sync.dma_start(ctx_out[r:r + 1, :], ctx_sb[:])

            for r in range(R):
                row = {
                    "mask": mask_sb[:, r * T:(r + 1) * T],
                    "scores": row_pool.tile([1, T], F32, tag="scores", name="scores_sb"),
                    "mx4": row_pool.tile([1, NTC], MM_DT, tag="mx4", name="mx4_sb"),
                }
                row_state[r] = row
                for c in range(NTC):
                    if r == 0 and c == 0:
                        xt_sb = xt_first
                    elif r == 0 and c == 1:
                        xt_sb = xt_second
                    else:
                        xt_sb = load_xt_chunk(r, c)
                    if c == 3:
                        phase_b_load(r, row)
                    phase_a_chunk(r, c, xt_sb, row)
                    # previous row's context matmuls slot in here, one chunk
                    # deep into this row, so PE never stalls on its softmax.
                    if r > 0 and c == 1:
                        phase_b_row(r - 1, row_state[r - 1])
                softmax_row(r, row)
            phase_b_row(R - 1, row_state[R - 1])

    if legalize:
        _legalize_waits(nc)
    return nc


def _get_program() -> bass.Bass:
    global _PROGRAM
    if _PROGRAM is None:
        _PROGRAM = build_program()
    return _PROGRAM


def make_in_maps(queries, encoder_output, lengths, v, Wq, Wk):
    """Host-side marshalling: shard batch across cores + layout shuffles."""
    queries = np.ascontiguousarray(np.asarray(queries), dtype=np.float32)
    encoder_output = np.ascontiguousarray(np.asarray(encoder_output), dtype=np.float32)
    lengths = np.asarray(lengths).astype(np.int64)
    v = np.asarray(v, dtype=np.float32)
    Wq = np.asarray(Wq, dtype=np.float32)
    Wk = np.asarray(Wk, dtype=np.float32)

    qp = queries[:, 0, :] @ Wq.T                                   # [N, U]
    xt_full = np.ascontiguousarray(encoder_output.transpose(0, 2, 1))  # [N, E, T]
    # wkt[p, ut, et, j] = Wk[ut*128+j, et*128+p]
    wkt = np.ascontiguousarray(Wk.reshape(UT, P, ET, P).transpose(3, 0, 2, 1))
    vt = np.ascontiguousarray(v.reshape(UT, P).T)                  # [P, UT]
    mask = np.where(
        np.arange(T)[None, :] >= lengths[:, None], MASK_NEG, np.float32(0.0)
    ).astype(np.float32)                                           # [N, T]

    in_maps = []
    for i in range(N_CORES):
        sl = slice(i * R, (i + 1) * R)
        qpt = np.ascontiguousarray(
            qp[sl].reshape(R, UT, P).transpose(2, 0, 1).reshape(P, R * UT)
        )
        in_maps.append({
            "xt": xt_full[sl],
            "xn": encoder_output[sl],
            "wkt": wkt,
            "vq": np.ascontiguousarray(np.concatenate([vt, qpt], axis=1)),
            "mask": np.ascontiguousarray(mask[sl].reshape(1, R * T)).astype(ml_dtypes.bfloat16),
        })
    return in_maps


def kernel(queries, encoder_output, lengths, v, Wq, Wk, _trace=False):
    global LAST_RESULTS
    in_maps = make_in_maps(queries, encoder_output, lengths, v, Wq, Wk)
    nc = _get_program()
    res = run_bass_kernel_spmd(
        nc, in_maps, core_ids=list(range(N_CORES)), trace=_trace
    )
    LAST_RESULTS = res
    contexts = np.concatenate(
        [res.results[i]["contexts"] for i in range(N_CORES)], axis=0
    )
    alignments = np.concatenate(
        [res.results[i]["alignments"] for i in range(N_CORES)], axis=0
    )
    return contexts, alignments


# revision 26
# speedup vs baseline: 1.2110x; 1.0216x over previous
"""Bahdanau attention on TRN2 — data-parallel over batch across 8 NeuronCores.

Math per batch row n (shapes: T=2048 encoder steps, E=U=1024):
    K_projT[u, t] = sum_e Wk[u, e] * X[n, t, e]          (big matmul, [U, T] layout)
    th[u, t]      = tanh(K_projT[u, t] + q_proj[n, u])   (ACT, per-partition bias)
    scores[t]     = sum_u v[u] * th[u, t]                (PE, v as 1-col stationary)
    a[t]          = softmax(scores + mask[n])            (mask additive -1e30)
    ctx[e]        = sum_t a[t] * X[n, t, e]              (PE, aT cols as stationary)

Host precomputes q_proj = queries @ Wq.T (tiny), the additive mask from
`lengths`, X transposed per row ([E, T]) so the contraction dim lands on
SBUF partitions, plus small layout shuffles of Wk / v / q_proj.
"""

import ml_dtypes
import numpy as np

import concourse.bass as bass
import concourse.mybir as mybir
import concourse.tile as tile
from concourse.bass_utils import run_bass_kernel_spmd

# Problem shape (hardcoded per contract; kernel.py must be self-contained).
N, T, D_ENC, D_DEC, U = 32, 2048, 1024, 1024, 1024
N_CORES = 8
R = N // N_CORES            # batch rows per core
P = 128                     # SBUF partitions
TC = 512                    # t-chunk = matmul moving free dim (fp32 max)
NTC = T // TC
ET = D_ENC // P             # e-tiles (contraction of the big matmul)
UT = U // P                 # u-tiles
TT = T // P                 # t-tiles (contraction of the context matmul)
EC = 512
NEC = D_ENC // EC

F32 = mybir.dt.float32
# PE matmul dtype. float32r = single-pass fp32 matmul (full rate at free
# dim >= 256); plain float32 = 2 half-speed passes (4x slower).
MM_DT = mybir.dt.float32r

AF = mybir.ActivationFunctionType
AX = mybir.AxisListType

MASK_NEG = np.float32(-1.0e30)

LAST_RESULTS = None         # BassKernelResults of the most recent run
_PROGRAM = None


def _mm(ap):
    return ap if ap.dtype == MM_DT else ap.bitcast(MM_DT)


def _legalize_waits(nc):
    """Several walrus instruction encodings (the self-loading fp32r matmul's
    S3_LW, Activation's S3D3_AC, ...) have a single sync-wait slot, but Tile
    sometimes emits 2+ waits on one instruction. Hoist the extra waits onto
    engine NoOps inserted just before the instruction — the engine's NX
    evaluates waits in program order, so gating is preserved. This covers
    HWDGE DMAs too: the issuing engine's sequencer writes the descriptor
    in program order, so a same-engine NoOp gates the transfer."""
    for f in nc.m.functions:
        for blk in f.blocks:
            insts = blk.instructions
            idx = 0
            while idx < len(insts):
                ins = insts[idx]
                if (
                    not isinstance(ins, mybir.InstCollectiveCompute)
                    and ins.engine is not None
                    and ins.sync_info is not None
                    and len(ins.sync_info.on_wait) > 1
                ):
                    waits = list(ins.sync_info.on_wait)
                    # one wait per NoOp — every ISA ctrl struct fits that
                    for w in waits[1:]:
                        nop = mybir.InstNoOp(
                            name=nc.get_next_instruction_name(), ins=[], outs=[]
                        )
                        nop.engine = ins.engine
                        nop.sync_info = mybir.SyncInfo(on_wait=[w], on_update=[])
                        insts.insert(idx, nop)
                        idx += 1
                    ins.sync_info = mybir.SyncInfo(
                        on_wait=[waits[0]], on_update=list(ins.sync_info.on_update)
                    )
                idx += 1


def build_program(legalize: bool = True) -> bass.Bass:
    nc = bass.Bass("TRN2")

    xt = nc.dram_tensor("xt", [R, D_ENC, T], F32, kind="ExternalInput").ap()
    xn = nc.dram_tensor("xn", [R, T, D_ENC], F32, kind="ExternalInput").ap()
    wkt = nc.dram_tensor("wkt", [P, UT, ET, P], F32, kind="ExternalInput").ap()
    vq = nc.dram_tensor("vq", [P, UT + R * UT + 1], F32, kind="ExternalInput").ap()
    mask = nc.dram_tensor("mask", [1, R * T], mybir.dt.bfloat16, kind="ExternalInput").ap()
    ctx_out = nc.dram_tensor("contexts", [R, D_ENC], F32, kind="ExternalOutput").ap()
    align_out = nc.dram_tensor("alignments", [R, T], F32, kind="ExternalOutput").ap()

    with tile.TileContext(nc) as tc:
        with (
            tc.tile_pool(name="const", bufs=1) as const_pool,
            tc.tile_pool(name="xtp", bufs=3) as xt_pool,
            tc.tile_pool(name="thp", bufs=4) as th_pool,
            tc.tile_pool(name="xnp", bufs=12) as xn_pool,
            tc.tile_pool(name="rowp", bufs=2) as row_pool,
            tc.tile_pool(name="psm", bufs=3, space="PSUM") as psum_m,
            tc.tile_pool(name="pss", bufs=2, space="PSUM") as psum_s,
            tc.tile_pool(name="psc", bufs=2, space="PSUM") as psum_c,
            tc.tile_pool(name="psj", bufs=1, space="PSUM") as psum_j,
            tc.tile_pool(name="dramp", bufs=2, space="DRAM") as dram_pool,
        ):
            # Replicated constants + startup interleave. All HWDGE DMAs
            # execute as one serial stream in issue order (each transfer
            # alone saturates ~430 GB/s), so ordering is everything: tiny
            # tensors first (they gate the tanh/score chain), then Wk
            # u-slices interleaved with the first xt chunks to match the
            # PE's consumption order.
            vq_sb = const_pool.tile([P, UT + R * UT + 1], MM_DT, tag="vq")
            nc.sync.dma_start(vq_sb[:], _mm(vq[:, :]))
            vt_sb = vq_sb[:, 0:UT]
            # same bits, fp32 view — the DMA copies bits, nothing is rounded
            qpt_sb = vq_sb[:, UT:UT + R * UT].bitcast(F32)
            one_sb = vq_sb[:, UT + R * UT:UT + R * UT + 1]   # constant 1.0
            mask_sb = const_pool.tile([1, R * T], mybir.dt.bfloat16, tag="mask")
            nc.sync.dma_start(mask_sb[:], mask[:, :])

            wk_sb = const_pool.tile([P, UT, ET, P], MM_DT, tag="wk")

            def load_wk(ut):
                nc.sync.dma_start(wk_sb[:, ut], _mm(wkt[:, ut]))

            def load_xt_part(xt_sb, r, c, ets):
                for et in ets:
                    nc.sync.dma_start(
                        xt_sb[:, et],
                        _mm(xt[r, et * P:(et + 1) * P, c * TC:(c + 1) * TC]),
                    )

            def load_xt_chunk(r, c):
                xt_sb = xt_pool.tile([P, ET, TC], MM_DT, tag="xt")
                load_xt_part(xt_sb, r, c, range(ET))
                return xt_sb

            load_wk(0)
            xt_first = xt_pool.tile([P, ET, TC], MM_DT, tag="xt", name="xt_sb")
            load_xt_part(xt_first, 0, 0, range(ET))
            load_wk(1)
            load_wk(2)
            load_wk(3)
            xt_second = xt_pool.tile([P, ET, TC], MM_DT, tag="xt", name="xt_sb")
            load_xt_part(xt_second, 0, 1, range(4))
            load_wk(4)
            load_wk(5)
            load_xt_part(xt_second, 0, 1, range(4, ET))
            load_wk(6)
            load_wk(7)

            row_state = {}

            def phase_a_chunk(r, c, xt_sb, row):
                sc_ps = psum_s.tile([1, TC], F32, tag="sc")
                ths = []
                for ut in range(UT):
                    ps = psum_m.tile([P, TC], F32, tag="kproj")
                    for et in range(ET):
                        nc.tensor.matmul(
                            ps[:],
                            wk_sb[:, ut, et],
                            xt_sb[:, et],
                            start=(et == 0),
                            stop=(et == ET - 1),
                        )
                    th = th_pool.tile([P, TC], MM_DT, tag="th")
                    nc.scalar.activation(
                        th[:], ps[:], AF.Tanh,
                        bias=qpt_sb[:, r * UT + ut:r * UT + ut + 1],
                    )
                    ths.append(th)
                    # score matmul for ut-1: one main group behind, so the
                    # tanh it waits on is already finished (no PE stall).
                    if ut > 0:
                        nc.tensor.matmul(
                            sc_ps[:], vt_sb[:, ut - 1:ut], _mm(ths[ut - 1][:]),
                            start=(ut == 1), stop=False,
                        )
                nc.tensor.matmul(
                    sc_ps[:], vt_sb[:, UT - 1:UT], _mm(ths[UT - 1][:]),
                    start=False, stop=True,
                )
                nc.vector.tensor_add(
                    row["scores"][:, c * TC:(c + 1) * TC], sc_ps[:],
                    row["mask"][:, c * TC:(c + 1) * TC],
                )
                # per-chunk max, so the end-of-row reduction is tiny and the
                # PE gap before the context matmuls stays under the HAM
                # re-throttle window.
                nc.vector.reduce_max(
                    row["mx4"][:, c:c + 1],
                    row["scores"][:, c * TC:(c + 1) * TC], axis=AX.X,
                )

            def keepalive(dep_ap):
                # tiny real matmul that reads the given (fp32r) tile: threads
                # a PE instruction through the softmax chain so the HAM
                # activity monitor never sees an idle window and the context
                # matmuls that follow run at full clock.
                jp = psum_j.tile([1, TC], F32, tag="junk", name="junk_ps")
                n = dep_ap.shape[-1]
                nc.tensor.matmul(
                    jp[:, 0:n], dep_ap[0:1, 0:1], dep_ap[0:1, :],
                    start=True, stop=True,
                )

            def softmax_row(r, row):
                mxn = row_pool.tile([1, 1], F32, tag="mxn")
                nc.vector.reduce_max(mxn[:], row["mx4"][:], axis=AX.X, negate=True)
                keepalive(row["mx4"][:])
                exp_sb = row_pool.tile([1, T], MM_DT, tag="exp")
                z4 = row_pool.tile([1, NTC], F32, tag="z4")
                bounce = dram_pool.tile([1, T], MM_DT, tag="bounce")
                at_sb = row_pool.tile([P, TT], MM_DT, tag="at")
                # exp -> DRAM bounce -> partition-scatter, pipelined per
                # 512-chunk: the first context matmuls only need at[:, 0:4],
                # so the PE resumes ~3 us earlier, and each chunk threads a
                # keepalive matmul so the PE clock never re-throttles.
                last = r == R - 1
                for c in range(NTC):
                    cs = slice(c * TC, (c + 1) * TC)
                    nc.scalar.activation(
                        exp_sb[:, cs], row["scores"][:, cs], AF.Exp,
                        bias=mxn[:], accum_out=z4[:, c:c + 1],
                    )
                    if last:
                        # The context matmuls are fully exposed after the
                        # final row, and each DMA hop costs ~3 us of
                        # completion latency. Transpose exp on the PE
                        # instead: 4 tiny transpose-matmuls per chunk into
                        # PSUM, one DVE copy out — at[:, 4c:4c+4] is ready
                        # ~1 us after its exp chunk, and the PE never idles.
                        tp_ps = psum_j.tile([P, TC // P], F32, tag="junk",
                                            name="tp_ps")
                        for j in range(TC // P):
                            tt = c * (TC // P) + j
                            nc.tensor.transpose(
                                tp_ps[:, j:j + 1],
                                exp_sb[:, tt * P:(tt + 1) * P].bitcast(F32),
                                one_sb[0:1, :].bitcast(F32),
                            )
                        nc.vector.tensor_copy(
                            at_sb[:, c * (TC // P):(c + 1) * (TC // P)], tp_ps[:]
                        )
                    else:
                        nc.sync.dma_start(bounce[:, cs], exp_sb[:, cs])
                        nc.sync.dma_start(
                            at_sb[:, c * (TC // P):(c + 1) * (TC // P)],
                            bounce[0, cs].rearrange("(j p) -> p j", p=P),
                        )
                        keepalive(exp_sb[:, cs])
                zsum = row_pool.tile([1, 1], F32, tag="z")
                nc.vector.reduce_sum(zsum[:], z4[:], axis=AX.X)
                rz = row_pool.tile([1, 1], F32, tag="rz")
                nc.vector.reciprocal(rz[:], zsum[:])
                align_sb = row["scores"]
                nc.scalar.activation(align_sb[:], exp_sb[:], AF.Copy, scale=rz[:])
                nc.sync.dma_start(align_out[r:r + 1, :], align_sb[:])
                row["at"], row["rz"] = at_sb, rz

            def phase_b_load(r, row):
                tiles = []
                for tt in range(TT):
                    xn_sb = xn_pool.tile([P, D_ENC], MM_DT, tag="xn")
                    nc.sync.dma_start(xn_sb[:], _mm(xn[r, tt * P:(tt + 1) * P, :]))
                    tiles.append(xn_sb)
                row["xn"] = tiles

            def phase_b_row(r, row):
                # ctx[e] = (1/Z) * sum_t exp[t] * X[t, e]
                ct_ps0 = psum_c.tile([1, EC], F32, tag="ctx")
                ct_ps1 = psum_c.tile([1, EC], F32, tag="ctx")
                at_sb, rz = row["at"], row["rz"]
                for tt in range(TT):
                    xn_sb = row["xn"][tt]
                    nc.tensor.matmul(
                        ct_ps0[:], at_sb[:, tt:tt + 1], xn_sb[:, 0:EC],
                        start=(tt == 0), stop=(tt == TT - 1),
                    )
                    nc.tensor.matmul(
                        ct_ps1[:], at_sb[:, tt:tt + 1], xn_sb[:, EC:2 * EC],
                        start=(tt == 0), stop=(tt == TT - 1),
                    )
                ctx_sb = row_pool.tile([1, D_ENC], F32, tag="ctx_sb", bufs=1)
                nc.scalar.activation(ctx_sb[:, 0:EC], ct_ps0[:], AF.Copy, scale=rz[:])
                nc.scalar.activation(ctx_sb[:, EC:], ct_ps1[:], AF.Copy, scale=rz[:])
                nc.sync.dma_start(ctx_out[r:r + 1, :], ctx_sb[:])

            for r in range(R):
                row = {
                    "mask": mask_sb[:, r * T:(r + 1) * T],
                    "scores": row_pool.tile([1, T], F32, tag="scores", name="scores_sb"),
                    "mx4": row_pool.tile([1, NTC], MM_DT, tag="mx4", name="mx4_sb"),
                }
                row_state[r] = row
                for c in range(NTC):
                    if r == 0 and c == 0:
                        xt_sb = xt_first
                    elif r == 0 and c == 1:
                        xt_sb = xt_second
                    else:
                        xt_sb = load_xt_chunk(r, c)
                    if c == 3:
                        phase_b_load(r, row)
                    phase_a_chunk(r, c, xt_sb, row)
                    # previous row's context matmuls slot in here, one chunk
                    # deep into this row, so PE never stalls on its softmax.
                    if r > 0 and c == 1:
                        phase_b_row(r - 1, row_state[r - 1])
                softmax_row(r, row)
            phase_b_row(R - 1, row_state[R - 1])

    if legalize:
        _legalize_waits(nc)
    return nc


def _get_program() -> bass.Bass:
    global _PROGRAM
    if _PROGRAM is None:
        _PROGRAM = build_program()
    return _PROGRAM


def make_in_maps(queries, encoder_output, lengths, v, Wq, Wk):
    """Host-side marshalling: shard batch across cores + layout shuffles."""
    queries = np.ascontiguousarray(np.asarray(queries), dtype=np.float32)
    encoder_output = np.ascontiguousarray(np.asarray(encoder_output), dtype=np.float32)
    lengths = np.asarray(lengths).astype(np.int64)
    v = np.asarray(v, dtype=np.float32)
    Wq = np.asarray(Wq, dtype=np.float32)
    Wk = np.asarray(Wk, dtype=np.float32)

    qp = queries[:, 0, :] @ Wq.T                                   # [N, U]
    xt_full = np.ascontiguousarray(encoder_output.transpose(0, 2, 1))  # [N, E, T]
    # wkt[p, ut, et, j] = Wk[ut*128+j, et*128+p]
    wkt = np.ascontiguousarray(Wk.reshape(UT, P, ET, P).transpose(3, 0, 2, 1))
    vt = np.ascontiguousarray(v.reshape(UT, P).T)                  # [P, UT]
    mask = np.where(
        np.arange(T)[None, :] >= lengths[:, None], MASK_NEG, np.float32(0.0)
    ).astype(np.float32)                                           # [N, T]

    in_maps = []
    for i in range(N_CORES):
        sl = slice(i * R, (i + 1) * R)
        qpt = np.ascontiguousarray(
            qp[sl].reshape(R, UT, P).transpose(2, 0, 1).reshape(P, R * UT)
        )
        in_maps.append({
            "xt": xt_full[sl],
            "xn": encoder_output[sl],
            "wkt": wkt,
            "vq": np.ascontiguousarray(np.concatenate(
                [vt, qpt, np.ones((P, 1), np.float32)], axis=1)),
            "mask": np.ascontiguousarray(mask[sl].reshape(1, R * T)).astype(ml_dtypes.bfloat16),
        })
    return in_maps


def kernel(queries, encoder_output, lengths, v, Wq, Wk, _trace=False):
    global LAST_RESULTS
    in_maps = make_in_maps(queries, encoder_output, lengths, v, Wq, Wk)
    nc = _get_program()
    res = run_bass_kernel_spmd(
        nc, in_maps, core_ids=list(range(N_CORES)), trace=_trace
    )
    LAST_RESULTS = res
    contexts = np.concatenate(
        [res.results[i]["contexts"] for i in range(N_CORES)], axis=0
    )
    alignments = np.concatenate(
        [res.results[i]["alignments"] for i in range(N_CORES)], axis=0
    )
    return contexts, alignments


# revision 27
# speedup vs baseline: 1.2129x; 1.0016x over previous
"""Bahdanau attention on TRN2 — data-parallel over batch across 8 NeuronCores.

Math per batch row n (shapes: T=2048 encoder steps, E=U=1024):
    K_projT[u, t] = sum_e Wk[u, e] * X[n, t, e]          (big matmul, [U, T] layout)
    th[u, t]      = tanh(K_projT[u, t] + q_proj[n, u])   (ACT, per-partition bias)
    scores[t]     = sum_u v[u] * th[u, t]                (PE, v as 1-col stationary)
    a[t]          = softmax(scores + mask[n])            (mask additive -1e30)
    ctx[e]        = sum_t a[t] * X[n, t, e]              (PE, aT cols as stationary)

Host precomputes q_proj = queries @ Wq.T (tiny), the additive mask from
`lengths`, X transposed per row ([E, T]) so the contraction dim lands on
SBUF partitions, plus small layout shuffles of Wk / v / q_proj.
"""

import ml_dtypes
import numpy as np

import concourse.bass as bass
import concourse.mybir as mybir
import concourse.tile as tile
from concourse.bass_utils import run_bass_kernel_spmd

# Problem shape (hardcoded per contract; kernel.py must be self-contained).
N, T, D_ENC, D_DEC, U = 32, 2048, 1024, 1024, 1024
N_CORES = 8
R = N // N_CORES            # batch rows per core
P = 128                     # SBUF partitions
TC = 512                    # t-chunk = matmul moving free dim (fp32 max)
NTC = T // TC
ET = D_ENC // P             # e-tiles (contraction of the big matmul)
UT = U // P                 # u-tiles
TT = T // P                 # t-tiles (contraction of the context matmul)
EC = 512
NEC = D_ENC // EC

F32 = mybir.dt.float32
# PE matmul dtype. float32r = single-pass fp32 matmul (full rate at free
# dim >= 256); plain float32 = 2 half-speed passes (4x slower).
MM_DT = mybir.dt.float32r

AF = mybir.ActivationFunctionType
AX = mybir.AxisListType

MASK_NEG = np.float32(-1.0e30)

LAST_RESULTS = None         # BassKernelResults of the most recent run
_PROGRAM = None


def _mm(ap):
    return ap if ap.dtype == MM_DT else ap.bitcast(MM_DT)


def _legalize_waits(nc):
    """Several walrus instruction encodings (the self-loading fp32r matmul's
    S3_LW, Activation's S3D3_AC, ...) have a single sync-wait slot, but Tile
    sometimes emits 2+ waits on one instruction. Hoist the extra waits onto
    engine NoOps inserted just before the instruction — the engine's NX
    evaluates waits in program order, so gating is preserved. This covers
    HWDGE DMAs too: the issuing engine's sequencer writes the descriptor
    in program order, so a same-engine NoOp gates the transfer."""
    for f in nc.m.functions:
        for blk in f.blocks:
            insts = blk.instructions
            idx = 0
            while idx < len(insts):
                ins = insts[idx]
                if (
                    not isinstance(ins, mybir.InstCollectiveCompute)
                    and ins.engine is not None
                    and ins.sync_info is not None
                    and len(ins.sync_info.on_wait) > 1
                ):
                    waits = list(ins.sync_info.on_wait)
                    # one wait per NoOp — every ISA ctrl struct fits that
                    for w in waits[1:]:
                        nop = mybir.InstNoOp(
                            name=nc.get_next_instruction_name(), ins=[], outs=[]
                        )
                        nop.engine = ins.engine
                        nop.sync_info = mybir.SyncInfo(on_wait=[w], on_update=[])
                        insts.insert(idx, nop)
                        idx += 1
                    ins.sync_info = mybir.SyncInfo(
                        on_wait=[waits[0]], on_update=list(ins.sync_info.on_update)
                    )
                idx += 1


def build_program(legalize: bool = True) -> bass.Bass:
    nc = bass.Bass("TRN2")

    xt = nc.dram_tensor("xt", [R, D_ENC, T], F32, kind="ExternalInput").ap()
    xn = nc.dram_tensor("xn", [R, T, D_ENC], F32, kind="ExternalInput").ap()
    wkt = nc.dram_tensor("wkt", [P, UT, ET, P], F32, kind="ExternalInput").ap()
    vq = nc.dram_tensor("vq", [P, UT + R * UT + 1], F32, kind="ExternalInput").ap()
    mask = nc.dram_tensor("mask", [1, R * T], mybir.dt.bfloat16, kind="ExternalInput").ap()
    ctx_out = nc.dram_tensor("contexts", [R, D_ENC], F32, kind="ExternalOutput").ap()
    align_out = nc.dram_tensor("alignments", [R, T], F32, kind="ExternalOutput").ap()

    with tile.TileContext(nc) as tc:
        with (
            tc.tile_pool(name="const", bufs=1) as const_pool,
            tc.tile_pool(name="xtp", bufs=3) as xt_pool,
            tc.tile_pool(name="thp", bufs=4) as th_pool,
            tc.tile_pool(name="xnp", bufs=12) as xn_pool,
            tc.tile_pool(name="rowp", bufs=2) as row_pool,
            tc.tile_pool(name="psm", bufs=3, space="PSUM") as psum_m,
            tc.tile_pool(name="pss", bufs=2, space="PSUM") as psum_s,
            tc.tile_pool(name="psc", bufs=2, space="PSUM") as psum_c,
            tc.tile_pool(name="psj", bufs=1, space="PSUM") as psum_j,
            tc.tile_pool(name="dramp", bufs=2, space="DRAM") as dram_pool,
        ):
            # Replicated constants + startup interleave. All HWDGE DMAs
            # execute as one serial stream in issue order (each transfer
            # alone saturates ~430 GB/s), so ordering is everything: tiny
            # tensors first (they gate the tanh/score chain), then Wk
            # u-slices interleaved with the first xt chunks to match the
            # PE's consumption order.
            vq_sb = const_pool.tile([P, UT + R * UT + 1], MM_DT, tag="vq")
            nc.sync.dma_start(vq_sb[:], _mm(vq[:, :]))
            vt_sb = vq_sb[:, 0:UT]
            # same bits, fp32 view — the DMA copies bits, nothing is rounded
            qpt_sb = vq_sb[:, UT:UT + R * UT].bitcast(F32)
            one_sb = vq_sb[:, UT + R * UT:UT + R * UT + 1]   # constant 1.0
            wk_sb = const_pool.tile([P, UT, ET, P], MM_DT, tag="wk")
            mask_sb = const_pool.tile([1, R * T], mybir.dt.bfloat16, tag="mask")

            def load_wk(ut):
                nc.sync.dma_start(wk_sb[:, ut], _mm(wkt[:, ut]))

            def load_xt_part(xt_sb, r, c, ets):
                for et in ets:
                    nc.sync.dma_start(
                        xt_sb[:, et],
                        _mm(xt[r, et * P:(et + 1) * P, c * TC:(c + 1) * TC]),
                    )

            def load_xt_chunk(r, c):
                xt_sb = xt_pool.tile([P, ET, TC], MM_DT, tag="xt")
                load_xt_part(xt_sb, r, c, range(ET))
                return xt_sb

            load_wk(0)
            xt_first = xt_pool.tile([P, ET, TC], MM_DT, tag="xt", name="xt_sb")
            load_xt_part(xt_first, 0, 0, range(ET))
            nc.sync.dma_start(mask_sb[:], mask[:, :])
            load_wk(1)
            load_wk(2)
            load_wk(3)
            xt_second = xt_pool.tile([P, ET, TC], MM_DT, tag="xt", name="xt_sb")
            load_xt_part(xt_second, 0, 1, range(4))
            load_wk(4)
            load_wk(5)
            load_xt_part(xt_second, 0, 1, range(4, ET))
            load_wk(6)
            load_wk(7)

            row_state = {}

            def phase_a_chunk(r, c, xt_sb, row):
                sc_ps = psum_s.tile([1, TC], F32, tag="sc")
                ths = []
                for ut in range(UT):
                    ps = psum_m.tile([P, TC], F32, tag="kproj")
                    for et in range(ET):
                        nc.tensor.matmul(
                            ps[:],
                            wk_sb[:, ut, et],
                            xt_sb[:, et],
                            start=(et == 0),
                            stop=(et == ET - 1),
                        )
                    th = th_pool.tile([P, TC], MM_DT, tag="th")
                    nc.scalar.activation(
                        th[:], ps[:], AF.Tanh,
                        bias=qpt_sb[:, r * UT + ut:r * UT + ut + 1],
                    )
                    ths.append(th)
                    # score matmul for ut-1: one main group behind, so the
                    # tanh it waits on is already finished (no PE stall).
                    if ut > 0:
                        nc.tensor.matmul(
                            sc_ps[:], vt_sb[:, ut - 1:ut], _mm(ths[ut - 1][:]),
                            start=(ut == 1), stop=False,
                        )
                nc.tensor.matmul(
                    sc_ps[:], vt_sb[:, UT - 1:UT], _mm(ths[UT - 1][:]),
                    start=False, stop=True,
                )
                nc.vector.tensor_add(
                    row["scores"][:, c * TC:(c + 1) * TC], sc_ps[:],
                    row["mask"][:, c * TC:(c + 1) * TC],
                )
                # per-chunk max, so the end-of-row reduction is tiny and the
                # PE gap before the context matmuls stays under the HAM
                # re-throttle window.
                nc.vector.reduce_max(
                    row["mx4"][:, c:c + 1],
                    row["scores"][:, c * TC:(c + 1) * TC], axis=AX.X,
                )

            def keepalive(dep_ap):
                # tiny real matmul that reads the given (fp32r) tile: threads
                # a PE instruction through the softmax chain so the HAM
                # activity monitor never sees an idle window and the context
                # matmuls that follow run at full clock.
                jp = psum_j.tile([1, TC], F32, tag="junk", name="junk_ps")
                n = dep_ap.shape[-1]
                nc.tensor.matmul(
                    jp[:, 0:n], dep_ap[0:1, 0:1], dep_ap[0:1, :],
                    start=True, stop=True,
                )

            def softmax_row(r, row):
                mxn = row_pool.tile([1, 1], F32, tag="mxn")
                nc.vector.reduce_max(mxn[:], row["mx4"][:], axis=AX.X, negate=True)
                keepalive(row["mx4"][:])
                exp_sb = row_pool.tile([1, T], MM_DT, tag="exp")
                z4 = row_pool.tile([1, NTC], F32, tag="z4")
                bounce = dram_pool.tile([1, T], MM_DT, tag="bounce")
                at_sb = row_pool.tile([P, TT], MM_DT, tag="at")
                # exp -> DRAM bounce -> partition-scatter, pipelined per
                # 512-chunk: the first context matmuls only need at[:, 0:4],
                # so the PE resumes ~3 us earlier, and each chunk threads a
                # keepalive matmul so the PE clock never re-throttles.
                last = r == R - 1
                for c in range(NTC):
                    cs = slice(c * TC, (c + 1) * TC)
                    nc.scalar.activation(
                        exp_sb[:, cs], row["scores"][:, cs], AF.Exp,
                        bias=mxn[:], accum_out=z4[:, c:c + 1],
                    )
                    if last:
                        # The context matmuls are fully exposed after the
                        # final row, and each DMA hop costs ~3 us of
                        # completion latency. Transpose exp on the PE
                        # instead: 4 tiny transpose-matmuls per chunk into
                        # PSUM, one DVE copy out — at[:, 4c:4c+4] is ready
                        # ~1 us after its exp chunk, and the PE never idles.
                        tp_ps = psum_j.tile([P, TC // P], F32, tag="junk",
                                            name="tp_ps")
                        for j in range(TC // P):
                            tt = c * (TC // P) + j
                            nc.tensor.transpose(
                                tp_ps[:, j:j + 1],
                                exp_sb[:, tt * P:(tt + 1) * P].bitcast(F32),
                                one_sb[0:1, :].bitcast(F32),
                            )
                        nc.vector.tensor_copy(
                            at_sb[:, c * (TC // P):(c + 1) * (TC // P)], tp_ps[:]
                        )
                    else:
                        nc.sync.dma_start(bounce[:, cs], exp_sb[:, cs])
                        nc.sync.dma_start(
                            at_sb[:, c * (TC // P):(c + 1) * (TC // P)],
                            bounce[0, cs].rearrange("(j p) -> p j", p=P),
                        )
                        keepalive(exp_sb[:, cs])
                zsum = row_pool.tile([1, 1], F32, tag="z")
                nc.vector.reduce_sum(zsum[:], z4[:], axis=AX.X)
                rz = row_pool.tile([1, 1], F32, tag="rz")
                nc.vector.reciprocal(rz[:], zsum[:])
                align_sb = row["scores"]
                nc.scalar.activation(align_sb[:], exp_sb[:], AF.Copy, scale=rz[:])
                nc.sync.dma_start(align_out[r:r + 1, :], align_sb[:])
                row["at"], row["rz"] = at_sb, rz

            def phase_b_load(r, row):
                tiles = []
                for tt in range(TT):
                    xn_sb = xn_pool.tile([P, D_ENC], MM_DT, tag="xn")
                    nc.sync.dma_start(xn_sb[:], _mm(xn[r, tt * P:(tt + 1) * P, :]))
                    tiles.append(xn_sb)
                row["xn"] = tiles

            def phase_b_row(r, row):
                # ctx[e] = (1/Z) * sum_t exp[t] * X[t, e]
                ct_ps0 = psum_c.tile([1, EC], F32, tag="ctx")
                ct_ps1 = psum_c.tile([1, EC], F32, tag="ctx")
                at_sb, rz = row["at"], row["rz"]
                for tt in range(TT):
                    xn_sb = row["xn"][tt]
                    nc.tensor.matmul(
                        ct_ps0[:], at_sb[:, tt:tt + 1], xn_sb[:, 0:EC],
                        start=(tt == 0), stop=(tt == TT - 1),
                    )
                    nc.tensor.matmul(
                        ct_ps1[:], at_sb[:, tt:tt + 1], xn_sb[:, EC:2 * EC],
                        start=(tt == 0), stop=(tt == TT - 1),
                    )
                ctx_sb = row_pool.tile([1, D_ENC], F32, tag="ctx_sb", bufs=1)
                nc.scalar.activation(ctx_sb[:, 0:EC], ct_ps0[:], AF.Copy, scale=rz[:])
                nc.scalar.activation(ctx_sb[:, EC:], ct_ps1[:], AF.Copy, scale=rz[:])
                nc.sync.dma_start(ctx_out[r:r + 1, :], ctx_sb[:])

            for r in range(R):
                row = {
                    "mask": mask_sb[:, r * T:(r + 1) * T],
                    "scores": row_pool.tile([1, T], F32, tag="scores", name="scores_sb"),
                    "mx4": row_pool.tile([1, NTC], MM_DT, tag="mx4", name="mx4_sb"),
                }
                row_state[r] = row
                for c in range(NTC):
                    if r == 0 and c == 0:
                        xt_sb = xt_first
                    elif r == 0 and c == 1:
                        xt_sb = xt_second
                    else:
                        xt_sb = load_xt_chunk(r, c)
                    if c == 3:
                        phase_b_load(r, row)
                    phase_a_chunk(r, c, xt_sb, row)
                    # previous row's context matmuls slot in here, one chunk
                    # deep into this row, so PE never stalls on its softmax.
                    if r > 0 and c == 1:
                        phase_b_row(r - 1, row_state[r - 1])
                softmax_row(r, row)
            phase_b_row(R - 1, row_state[R - 1])

    if legalize:
        _legalize_waits(nc)
    return nc


def _get_program() -> bass.Bass:
    global _PROGRAM
    if _PROGRAM is None:
        _PROGRAM = build_program()
    return _PROGRAM


def make_in_maps(queries, encoder_output, lengths, v, Wq, Wk):
    """Host-side marshalling: shard batch across cores + layout shuffles."""
    queries = np.ascontiguousarray(np.asarray(queries), dtype=np.float32)
    encoder_output = np.ascontiguousarray(np.asarray(encoder_output), dtype=np.float32)
    lengths = np.asarray(lengths).astype(np.int64)
    v = np.asarray(v, dtype=np.float32)
    Wq = np.asarray(Wq, dtype=np.float32)
    Wk = np.asarray(Wk, dtype=np.float32)

    qp = queries[:, 0, :] @ Wq.T                                   # [N, U]
    xt_full = np.ascontiguousarray(encoder_output.transpose(0, 2, 1))  # [N, E, T]
    # wkt[p, ut, et, j] = Wk[ut*128+j, et*128+p]
    wkt = np.ascontiguousarray(Wk.reshape(UT, P, ET, P).transpose(3, 0, 2, 1))
    vt = np.ascontiguousarray(v.reshape(UT, P).T)                  # [P, UT]
    mask = np.where(
        np.arange(T)[None, :] >= lengths[:, None], MASK_NEG, np.float32(0.0)
    ).astype(np.float32)                                           # [N, T]

    in_maps = []
    for i in range(N_CORES):
        sl = slice(i * R, (i + 1) * R)
        qpt = np.ascontiguousarray(
            qp[sl].reshape(R, UT, P).transpose(2, 0, 1).reshape(P, R * UT)
        )
        in_maps.append({
            "xt": xt_full[sl],
            "xn": encoder_output[sl],
            "wkt": wkt,
            "vq": np.ascontiguousarray(np.concatenate(
                [vt, qpt, np.ones((P, 1), np.float32)], axis=1)),
            "mask": np.ascontiguousarray(mask[sl].reshape(1, R * T)).astype(ml_dtypes.bfloat16),
        })
    return in_maps


def kernel(queries, encoder_output, lengths, v, Wq, Wk, _trace=False):
    global LAST_RESULTS
    in_maps = make_in_maps(queries, encoder_output, lengths, v, Wq, Wk)
    nc = _get_program()
    res = run_bass_kernel_spmd(
        nc, in_maps, core_ids=list(range(N_CORES)), trace=_trace
    )
    LAST_RESULTS = res
    contexts = np.concatenate(
        [res.results[i]["contexts"] for i in range(N_CORES)], axis=0
    )
    alignments = np.concatenate(
        [res.results[i]["alignments"] for i in range(N_CORES)], axis=0
    )
    return contexts, alignments


# revision 30
# speedup vs baseline: 1.2233x; 1.0086x over previous
"""Bahdanau attention on TRN2 — data-parallel over batch across 8 NeuronCores.

Math per batch row n (shapes: T=2048 encoder steps, E=U=1024):
    K_projT[u, t] = sum_e Wk[u, e] * X[n, t, e]          (big matmul, [U, T] layout)
    th[u, t]      = tanh(K_projT[u, t] + q_proj[n, u])   (ACT, per-partition bias)
    scores[t]     = sum_u v[u] * th[u, t]                (PE, v as 1-col stationary)
    a[t]          = softmax(scores + mask[n])            (mask additive -1e30)
    ctx[e]        = sum_t a[t] * X[n, t, e]              (PE, aT cols as stationary)

Host precomputes q_proj = queries @ Wq.T (tiny), the additive mask from
`lengths`, X transposed per row ([E, T]) so the contraction dim lands on
SBUF partitions, plus small layout shuffles of Wk / v / q_proj.
"""

import ml_dtypes
import numpy as np

import concourse.bass as bass
import concourse.mybir as mybir
import concourse.tile as tile
from concourse.bass_utils import run_bass_kernel_spmd

# Problem shape (hardcoded per contract; kernel.py must be self-contained).
N, T, D_ENC, D_DEC, U = 32, 2048, 1024, 1024, 1024
N_CORES = 8
R = N // N_CORES            # batch rows per core
P = 128                     # SBUF partitions
TC = 512                    # t-chunk = matmul moving free dim (fp32 max)
NTC = T // TC
ET = D_ENC // P             # e-tiles (contraction of the big matmul)
UT = U // P                 # u-tiles
TT = T // P                 # t-tiles (contraction of the context matmul)
EC = 512
NEC = D_ENC // EC

F32 = mybir.dt.float32
# PE matmul dtype. float32r = single-pass fp32 matmul (full rate at free
# dim >= 256); plain float32 = 2 half-speed passes (4x slower).
MM_DT = mybir.dt.float32r

AF = mybir.ActivationFunctionType
AX = mybir.AxisListType

MASK_NEG = np.float32(-1.0e30)

LAST_RESULTS = None         # BassKernelResults of the most recent run
_PROGRAM = None


def _mm(ap):
    return ap if ap.dtype == MM_DT else ap.bitcast(MM_DT)


def _legalize_waits(nc):
    """Several walrus instruction encodings (the self-loading fp32r matmul's
    S3_LW, Activation's S3D3_AC, ...) have a single sync-wait slot, but Tile
    sometimes emits 2+ waits on one instruction. Hoist the extra waits onto
    engine NoOps inserted just before the instruction — the engine's NX
    evaluates waits in program order, so gating is preserved. This covers
    HWDGE DMAs too: the issuing engine's sequencer writes the descriptor
    in program order, so a same-engine NoOp gates the transfer."""
    for f in nc.m.functions:
        for blk in f.blocks:
            insts = blk.instructions
            idx = 0
            while idx < len(insts):
                ins = insts[idx]
                if (
                    not isinstance(ins, mybir.InstCollectiveCompute)
                    and ins.engine is not None
                    and ins.sync_info is not None
                    and len(ins.sync_info.on_wait) > 1
                ):
                    waits = list(ins.sync_info.on_wait)
                    # one wait per NoOp — every ISA ctrl struct fits that
                    for w in waits[1:]:
                        nop = mybir.InstNoOp(
                            name=nc.get_next_instruction_name(), ins=[], outs=[]
                        )
                        nop.engine = ins.engine
                        nop.sync_info = mybir.SyncInfo(on_wait=[w], on_update=[])
                        insts.insert(idx, nop)
                        idx += 1
                    ins.sync_info = mybir.SyncInfo(
                        on_wait=[waits[0]], on_update=list(ins.sync_info.on_update)
                    )
                idx += 1


def build_program(legalize: bool = True) -> bass.Bass:
    nc = bass.Bass("TRN2")

    xt = nc.dram_tensor("xt", [R, D_ENC, T], F32, kind="ExternalInput").ap()
    xn = nc.dram_tensor("xn", [R, T, D_ENC], F32, kind="ExternalInput").ap()
    wkt = nc.dram_tensor("wkt", [P, UT, ET, P], F32, kind="ExternalInput").ap()
    vq = nc.dram_tensor("vq", [P, UT + R * UT + 1], F32, kind="ExternalInput").ap()
    mask = nc.dram_tensor("mask", [1, R * T], mybir.dt.bfloat16, kind="ExternalInput").ap()
    ctx_out = nc.dram_tensor("contexts", [R, D_ENC], F32, kind="ExternalOutput").ap()
    align_out = nc.dram_tensor("alignments", [R, T], F32, kind="ExternalOutput").ap()

    with tile.TileContext(nc) as tc:
        with (
            tc.tile_pool(name="const", bufs=1) as const_pool,
            tc.tile_pool(name="xtp", bufs=3) as xt_pool,
            tc.tile_pool(name="thp", bufs=4) as th_pool,
            tc.tile_pool(name="xnp", bufs=12) as xn_pool,
            tc.tile_pool(name="rowp", bufs=2) as row_pool,
            tc.tile_pool(name="psm", bufs=3, space="PSUM") as psum_m,
            tc.tile_pool(name="pss", bufs=2, space="PSUM") as psum_s,
            tc.tile_pool(name="psc", bufs=2, space="PSUM") as psum_c,
            tc.tile_pool(name="psj", bufs=1, space="PSUM") as psum_j,
            tc.tile_pool(name="dramp", bufs=2, space="DRAM") as dram_pool,
        ):
            # Replicated constants + startup interleave. All HWDGE DMAs
            # execute as one serial stream in issue order (each transfer
            # alone saturates ~430 GB/s), so ordering is everything: tiny
            # tensors first (they gate the tanh/score chain), then Wk
            # u-slices interleaved with the first xt chunks to match the
            # PE's consumption order.
            vq_sb = const_pool.tile([P, UT + R * UT + 1], MM_DT, tag="vq")
            nc.sync.dma_start(vq_sb[:], _mm(vq[:, :]))
            vt_sb = vq_sb[:, 0:UT]
            # same bits, fp32 view — the DMA copies bits, nothing is rounded
            qpt_sb = vq_sb[:, UT:UT + R * UT].bitcast(F32)
            one_sb = vq_sb[:, UT + R * UT:UT + R * UT + 1]   # constant 1.0
            mask_sb = const_pool.tile([1, R * T], mybir.dt.bfloat16, tag="mask")
            nc.sync.dma_start(mask_sb[:], mask[:, :])

            wk_sb = const_pool.tile([P, UT, ET, P], MM_DT, tag="wk")

            def load_wk(ut):
                nc.sync.dma_start(wk_sb[:, ut], _mm(wkt[:, ut]))

            def load_xt_part(xt_sb, r, c, ets):
                for et in ets:
                    nc.sync.dma_start(
                        xt_sb[:, et],
                        _mm(xt[r, et * P:(et + 1) * P, c * TC:(c + 1) * TC]),
                    )

            def load_xt_chunk(r, c):
                xt_sb = xt_pool.tile([P, ET, TC], MM_DT, tag="xt")
                load_xt_part(xt_sb, r, c, range(ET))
                return xt_sb

            load_wk(0)
            xt_first = xt_pool.tile([P, ET, TC], MM_DT, tag="xt", name="xt_sb")
            load_xt_part(xt_first, 0, 0, range(ET))
            load_wk(1)
            load_wk(2)
            load_wk(3)
            xt_second = xt_pool.tile([P, ET, TC], MM_DT, tag="xt", name="xt_sb")
            load_xt_part(xt_second, 0, 1, range(4))
            load_wk(4)
            load_wk(5)
            load_xt_part(xt_second, 0, 1, range(4, ET))
            load_wk(6)
            load_wk(7)

            row_state = {}

            def phase_a_chunk(r, c, xt_sb, row):
                sc_ps = psum_s.tile([1, TC], F32, tag="sc")
                ths = []
                for ut in range(UT):
                    ps = psum_m.tile([P, TC], F32, tag="kproj")
                    for et in range(ET):
                        nc.tensor.matmul(
                            ps[:],
                            wk_sb[:, ut, et],
                            xt_sb[:, et],
                            start=(et == 0),
                            stop=(et == ET - 1),
                        )
                    th = th_pool.tile([P, TC], MM_DT, tag="th")
                    nc.scalar.activation(
                        th[:], ps[:], AF.Tanh,
                        bias=qpt_sb[:, r * UT + ut:r * UT + ut + 1],
                    )
                    ths.append(th)
                    # score matmul for ut-1: one main group behind, so the
                    # tanh it waits on is already finished (no PE stall).
                    if ut > 0:
                        nc.tensor.matmul(
                            sc_ps[:], vt_sb[:, ut - 1:ut], _mm(ths[ut - 1][:]),
                            start=(ut == 1), stop=False,
                        )
                nc.tensor.matmul(
                    sc_ps[:], vt_sb[:, UT - 1:UT], _mm(ths[UT - 1][:]),
                    start=False, stop=True,
                )
                nc.vector.tensor_add(
                    row["scores"][:, c * TC:(c + 1) * TC], sc_ps[:],
                    row["mask"][:, c * TC:(c + 1) * TC],
                )
                # per-chunk max, so the end-of-row reduction is tiny and the
                # PE gap before the context matmuls stays under the HAM
                # re-throttle window.
                nc.vector.reduce_max(
                    row["mx4"][:, c:c + 1],
                    row["scores"][:, c * TC:(c + 1) * TC], axis=AX.X,
                )

            def keepalive(dep_ap):
                # tiny real matmul that reads the given (fp32r) tile: threads
                # a PE instruction through the softmax chain so the HAM
                # activity monitor never sees an idle window and the context
                # matmuls that follow run at full clock.
                jp = psum_j.tile([1, TC], F32, tag="junk", name="junk_ps")
                n = dep_ap.shape[-1]
                nc.tensor.matmul(
                    jp[:, 0:n], dep_ap[0:1, 0:1], dep_ap[0:1, :],
                    start=True, stop=True,
                )

            def softmax_row(r, row):
                last = r == R - 1
                mxn = row_pool.tile([1, 1], F32, tag="mxn")
                nc.vector.reduce_max(mxn[:], row["mx4"][:], axis=AX.X, negate=True)
                if last:
                    keepalive(row["mx4"][:])
                exp_sb = row_pool.tile([1, T], MM_DT, tag="exp")
                z4 = row_pool.tile([1, NTC], F32, tag="z4")
                bounce = dram_pool.tile([1, T], MM_DT, tag="bounce")
                at_sb = row_pool.tile([P, TT], MM_DT, tag="at")
                # exp -> DRAM bounce -> partition-scatter, pipelined per
                # 512-chunk: the first context matmuls only need at[:, 0:4],
                # so the PE resumes ~3 us earlier, and each chunk threads a
                # keepalive matmul so the PE clock never re-throttles.
                for c in range(NTC):
                    cs = slice(c * TC, (c + 1) * TC)
                    nc.scalar.activation(
                        exp_sb[:, cs], row["scores"][:, cs], AF.Exp,
                        bias=mxn[:], accum_out=z4[:, c:c + 1],
                    )
                    if last:
                        # The context matmuls are fully exposed after the
                        # final row, and each DMA hop costs ~3 us of
                        # completion latency. Transpose exp on the PE
                        # instead: 4 tiny transpose-matmuls per chunk into
                        # PSUM, one DVE copy out — at[:, 4c:4c+4] is ready
                        # ~1 us after its exp chunk, and the PE never idles.
                        tp_ps = psum_j.tile([P, TC // P], F32, tag="junk",
                                            name="tp_ps")
                        for j in range(TC // P):
                            tt = c * (TC // P) + j
                            nc.tensor.transpose(
                                tp_ps[:, j:j + 1],
                                exp_sb[:, tt * P:(tt + 1) * P].bitcast(F32),
                                one_sb[0:1, :].bitcast(F32),
                            )
                        nc.vector.tensor_copy(
                            at_sb[:, c * (TC // P):(c + 1) * (TC // P)], tp_ps[:]
                        )
                    else:
                        nc.sync.dma_start(bounce[:, cs], exp_sb[:, cs])
                        nc.sync.dma_start(
                            at_sb[:, c * (TC // P):(c + 1) * (TC // P)],
                            bounce[0, cs].rearrange("(j p) -> p j", p=P),
                        )
                zsum = row_pool.tile([1, 1], F32, tag="z")
                nc.vector.reduce_sum(zsum[:], z4[:], axis=AX.X)
                rz = row_pool.tile([1, 1], F32, tag="rz")
                nc.vector.reciprocal(rz[:], zsum[:])
                align_sb = row["scores"]
                nc.scalar.activation(align_sb[:], exp_sb[:], AF.Copy, scale=rz[:])
                nc.sync.dma_start(align_out[r:r + 1, :], align_sb[:])
                row["at"], row["rz"] = at_sb, rz

            def phase_b_load(r, row):
                tiles = []
                for tt in range(TT):
                    xn_sb = xn_pool.tile([P, D_ENC], MM_DT, tag="xn")
                    nc.sync.dma_start(xn_sb[:], _mm(xn[r, tt * P:(tt + 1) * P, :]))
                    tiles.append(xn_sb)
                row["xn"] = tiles

            def phase_b_row(r, row):
                # ctx[e] = (1/Z) * sum_t exp[t] * X[t, e]
                ct_ps0 = psum_c.tile([1, EC], F32, tag="ctx")
                ct_ps1 = psum_c.tile([1, EC], F32, tag="ctx")
                at_sb, rz = row["at"], row["rz"]
                for tt in range(TT):
                    xn_sb = row["xn"][tt]
                    nc.tensor.matmul(
                        ct_ps0[:], at_sb[:, tt:tt + 1], xn_sb[:, 0:EC],
                        start=(tt == 0), stop=(tt == TT - 1),
                    )
                    nc.tensor.matmul(
                        ct_ps1[:], at_sb[:, tt:tt + 1], xn_sb[:, EC:2 * EC],
                        start=(tt == 0), stop=(tt == TT - 1),
                    )
                ctx_sb = row_pool.tile([1, D_ENC], F32, tag="ctx_sb", bufs=1)
                nc.scalar.activation(ctx_sb[:, 0:EC], ct_ps0[:], AF.Copy, scale=rz[:])
                nc.scalar.activation(ctx_sb[:, EC:], ct_ps1[:], AF.Copy, scale=rz[:])
                nc.sync.dma_start(ctx_out[r:r + 1, :], ctx_sb[:])

            for r in range(R):
                row = {
                    "mask": mask_sb[:, r * T:(r + 1) * T],
                    "scores": row_pool.tile([1, T], F32, tag="scores", name="scores_sb"),
                    "mx4": row_pool.tile([1, NTC], MM_DT, tag="mx4", name="mx4_sb"),
                }
                row_state[r] = row
                for c in range(NTC):
                    if r == 0 and c == 0:
                        xt_sb = xt_first
                    elif r == 0 and c == 1:
                        xt_sb = xt_second
                    else:
                        xt_sb = load_xt_chunk(r, c)
                    if c == 3:
                        phase_b_load(r, row)
                    phase_a_chunk(r, c, xt_sb, row)
                    # previous row's context matmuls slot in here, one chunk
                    # deep into this row, so PE never stalls on its softmax.
                    if r > 0 and c == 1:
                        phase_b_row(r - 1, row_state[r - 1])
                softmax_row(r, row)
            phase_b_row(R - 1, row_state[R - 1])

    if legalize:
        _legalize_waits(nc)
    return nc


def _get_program() -> bass.Bass:
    global _PROGRAM
    if _PROGRAM is None:
        _PROGRAM = build_program()
    return _PROGRAM


def make_in_maps(queries, encoder_output, lengths, v, Wq, Wk):
    """Host-side marshalling: shard batch across cores + layout shuffles."""
    queries = np.ascontiguousarray(np.asarray(queries), dtype=np.float32)
    encoder_output = np.ascontiguousarray(np.asarray(encoder_output), dtype=np.float32)
    lengths = np.asarray(lengths).astype(np.int64)
    v = np.asarray(v, dtype=np.float32)
    Wq = np.asarray(Wq, dtype=np.float32)
    Wk = np.asarray(Wk, dtype=np.float32)

    qp = queries[:, 0, :] @ Wq.T                                   # [N, U]
    xt_full = np.ascontiguousarray(encoder_output.transpose(0, 2, 1))  # [N, E, T]
    # wkt[p, ut, et, j] = Wk[ut*128+j, et*128+p]
    wkt = np.ascontiguousarray(Wk.reshape(UT, P, ET, P).transpose(3, 0, 2, 1))
    vt = np.ascontiguousarray(v.reshape(UT, P).T)                  # [P, UT]
    mask = np.where(
        np.arange(T)[None, :] >= lengths[:, None], MASK_NEG, np.float32(0.0)
    ).astype(np.float32)                                           # [N, T]

    in_maps = []
    for i in range(N_CORES):
        sl = slice(i * R, (i + 1) * R)
        qpt = np.ascontiguousarray(
            qp[sl].reshape(R, UT, P).transpose(2, 0, 1).reshape(P, R * UT)
        )
        in_maps.append({
            "xt": xt_full[sl],
            "xn": encoder_output[sl],
            "wkt": wkt,
            "vq": np.ascontiguousarray(np.concatenate(
                [vt, qpt, np.ones((P, 1), np.float32)], axis=1)),
            "mask": np.ascontiguousarray(mask[sl].reshape(1, R * T)).astype(ml_dtypes.bfloat16),
        })
    return in_maps


def kernel(queries, encoder_output, lengths, v, Wq, Wk, _trace=False):
    global LAST_RESULTS
    in_maps = make_in_maps(queries, encoder_output, lengths, v, Wq, Wk)
    nc = _get_program()
    res = run_bass_kernel_spmd(
        nc, in_maps, core_ids=list(range(N_CORES)), trace=_trace
    )
    LAST_RESULTS = res
    contexts = np.concatenate(
        [res.results[i]["contexts"] for i in range(N_CORES)], axis=0
    )
    alignments = np.concatenate(
        [res.results[i]["alignments"] for i in range(N_CORES)], axis=0
    )
    return contexts, alignments
